# revision 1
# baseline (speedup 1.0000x reference)
"""Trainium2 Bass kernel for an equivariant GNN message-passing layer.

Full inputs in, full output out. 8-way owner-computes sharding by edge target
node (col). Per-edge squared distances are precomputed on host; the device
computes, per core c (nodes [c*S, (c+1)*S)):

  A-table:  At[n]   = emb[n] @ W1            (bf16, DRAM, slot-permuted rows)
  B-window: Bres[w] = emb_shard[w] @ W2 + b  (bf16, SBUF resident)
  msg[e]    = relu(At[row_e] + Bres[col_e] + dist_e * w_d)
  aggrT     = one-hot scatter-sum of msg by col      [128, S_pad] bf16
  outT      = Wres^T emb^T + relu(Wu1^T emb^T + Wu2^T aggrT + b_upd)

with W1 = W_msg[:128], W2 = W_msg[128:256], w_d = W_msg[256].

At rows are fetched per edge with gpsimd dma_gather (256B rows, int16
indices). The table is split in two halves so slot ids fit int16; within a
half, node r maps to slot (r%128)*C + r//128 (C=196 chunks) so the table
build writes 4KB-contiguous per-partition spans. All matmuls are bf16
(fp32 matmuls double-pump on TRN2). The output is produced transposed
[128, S_pad]; the host transposes back.
"""

import sys

for _p in ("/opt/trn_rl_repo",):
    if _p not in sys.path:
        sys.path.insert(0, _p)

import numpy as np
import ml_dtypes

import concourse.bacc as bacc
import concourse.bass as bass
import concourse.mybir as mybir
import concourse.tile as tile
from concourse.bass_utils import run_bass_kernel_spmd

F32 = mybir.dt.float32
BF16 = mybir.dt.bfloat16
I16 = mybir.dt.int16
BF = ml_dtypes.bfloat16

H = 128          # hidden/in channels (hardcoded for this problem)
RMAX = 8         # max 128-edge tiles per gather run


# --------------------------------------------------------------------------
# host-side prep
# --------------------------------------------------------------------------

def host_prep(node_embed, node_pos, W_res, W_msg, b_msg, W_upd, b_upd,
              edge_index, n_cores):
    N, C_in = node_embed.shape
    assert C_in == H and W_msg.shape == (2 * H + 1, H)
    assert N % n_cores == 0
    S = N // n_cores
    n_win = -(-S // 128)
    S_pad = n_win * 128
    N_pad = -(-N // 256) * 256
    N_half = N_pad // 2
    CH = N_half // 128                  # chunks per half (196)

    row = np.asarray(edge_index[0], dtype=np.int64)
    col = np.asarray(edge_index[1], dtype=np.int64)
    pos = np.asarray(node_pos, dtype=np.float32)
    diff = pos[row] - pos[col]
    dist = np.sum(diff * diff, axis=1).astype(np.float32)   # [E]

    core_of = col // S

    per_core = []
    counts = np.zeros((n_cores, 2, n_win), dtype=np.int64)
    for c in range(n_cores):
        sel = np.nonzero(core_of == c)[0]
        lc = col[sel] - c * S
        w = lc // 128
        hf = (row[sel] >= N_half).astype(np.int64)
        order = np.lexsort((w, hf))
        sel, w, hf = sel[order], w[order], hf[order]
        cw = (lc[order] % 128).astype(np.float32)
        np.add.at(counts[c], (hf, w), 1)
        per_core.append((sel, cw))

    tiles_hw = -(-counts.max(axis=0) // 128)            # [2, n_win]
    win_of_tile, half_of_tile = [], []
    for hf in (0, 1):
        for w in range(n_win):
            win_of_tile += [w] * int(tiles_hw[hf, w])
            half_of_tile += [hf] * int(tiles_hw[hf, w])
    T = len(win_of_tile)
    first_wp, last_wp = {}, {}
    for t, (w, hf) in enumerate(zip(win_of_tile, half_of_tile)):
        first_wp.setdefault((w, hf), t)
        last_wp[(w, hf)] = t

    # gather runs: consecutive same-half tiles, capped at RMAX
    runs = []
    t = 0
    while t < T:
        hf = half_of_tile[t]
        L = 1
        while (t + L < T and half_of_tile[t + L] == hf and L < RMAX):
            L += 1
        runs.append((t, L, hf))
        t += L
    R = len(runs)

    tile_base = {}
    b = 0
    for hf in (0, 1):
        for w in range(n_win):
            tile_base[(hf, w)] = b
            b += int(tiles_hw[hf, w])

    # per-core index/side arrays
    gidx_all, colp_all, colf_all, distT_all = [], [], [], []
    for c in range(n_cores):
        sel, cw = per_core[c]
        slots16 = np.zeros((T, 128), dtype=np.int16)
        colp = np.full((128, T), -1.0, dtype=np.float32)
        distp = np.zeros((T, 128), dtype=np.float32)
        start = 0
        for hf in (0, 1):
            for w in range(n_win):
                cnt = int(counts[c, hf, w])
                if cnt:
                    idx = np.arange(cnt)
                    t_loc = tile_base[(hf, w)] + idx // 128
                    lane = idx % 128
                    r = row[sel[start:start + cnt]] - hf * N_half
                    slot = (r % 128) * CH + r // 128
                    slots16[t_loc, lane] = slot.astype(np.int16)
                    distp[t_loc, lane] = dist[sel[start:start + cnt]]
                    start += cnt
                    colp[lane, t_loc] = cw[start - cnt:start]
        colf = np.full((R, RMAX * 128), -1.0, dtype=np.float32)
        distT = np.zeros((R, RMAX, 128), dtype=np.float32)
        gidx16 = np.zeros((R, 128, RMAX * 8), dtype=np.int16)
        for ri, (t0, L, hf) in enumerate(runs):
            flat = slots16[t0:t0 + L].reshape(L * 128)
            wrap = flat.reshape(-1, 16).T
            for rep in range(8):
                gidx16[ri, rep * 16:(rep + 1) * 16, :L * 8] = wrap
            for k in range(L):
                colf[ri, k * 128:(k + 1) * 128] = colp[:, t0 + k]
                distT[ri, k] = distp[t0 + k]
        gidx_all.append(np.ascontiguousarray(gidx16.transpose(1, 0, 2)))
        colp_all.append(colp.astype(BF))
        colf_all.append(colf.astype(BF))
        distT_all.append(distT.astype(BF))

    # replicated tensors
    emb = np.asarray(node_embed, dtype=np.float32)
    embT_full = np.zeros((H, N_pad), dtype=BF)
    embT_full[:, :N] = emb.T.astype(BF)

    iota = np.arange(128, dtype=np.float32)
    W_msg = np.asarray(W_msg, dtype=np.float32)
    W_upd = np.asarray(W_upd, dtype=np.float32)
    # three replicated row-blocks (partitions 0-7/8-15/16-23) so the dist
    # lhsT can sit at any of the 3 rotating dT partition bases
    wdiag = np.zeros((3 * RMAX, RMAX * 128), dtype=np.float32)
    for j in range(3):
        for k in range(RMAX):
            wdiag[j * RMAX + k, k * 128:(k + 1) * 128] = W_msg[2 * H]
    repl = {
        "W1": np.ascontiguousarray(W_msg[:H]).astype(BF),
        "W2": np.ascontiguousarray(W_msg[H:2 * H]).astype(BF),
        "wdiag": wdiag.astype(BF),
        "bmsg_row": np.asarray(b_msg, dtype=np.float32).reshape(1, H).astype(BF),
        "W_res": np.asarray(W_res, dtype=np.float32).astype(BF),
        "Wu1": np.ascontiguousarray(W_upd[:H]).astype(BF),
        "Wu2": np.ascontiguousarray(W_upd[H:]).astype(BF),
        "bupd_col": np.asarray(b_upd, dtype=np.float32).reshape(H, 1),
        "iota_p": iota.reshape(128, 1).copy(),
        "iota_rep": np.tile(iota.reshape(1, 128), (128, 1)).astype(BF),
        "ones_bf": np.ones((128, 128), np.float32).astype(BF),
        "embT_full": embT_full,
    }

    in_maps = []
    for c in range(n_cores):
        shardT = np.zeros((H, S_pad), dtype=BF)
        shardT[:, :S] = emb[c * S:(c + 1) * S].T.astype(BF)
        m = dict(repl)
        m["emb_shardT"] = shardT
        m["gidx16"] = gidx_all[c]
        m["colp"] = colp_all[c]
        m["colf"] = colf_all[c]
        m["distT"] = distT_all[c]
        in_maps.append(m)

    cfg = dict(N=N, N_pad=N_pad, N_half=N_half, CH=CH, S=S, S_pad=S_pad,
               n_win=n_win, R=R, T=T, runs=runs, win_of_tile=win_of_tile,
               half_of_tile=half_of_tile, first_wp=first_wp, last_wp=last_wp,
               n_cores=n_cores)
    return cfg, in_maps


# --------------------------------------------------------------------------
# device program
# --------------------------------------------------------------------------

def build_program(cfg, debug=False, fake_gather=False, skip_edges=False):
    N_half, CH, S_pad, n_win, R, T = (cfg["N_half"], cfg["CH"], cfg["S_pad"],
                                      cfg["n_win"], cfg["R"], cfg["T"])
    runs = cfg["runs"]
    win_of_tile = cfg["win_of_tile"]
    first_wp, last_wp = cfg["first_wp"], cfg["last_wp"]

    nc = bacc.Bacc("TRN2", target_bir_lowering=False, debug=debug,
                   num_devices=cfg["n_cores"])

    din = lambda n, s, dt: nc.dram_tensor(n, s, dt, kind="ExternalInput")
    embT_full = din("embT_full", [H, cfg["N_pad"]], BF16)
    W1 = din("W1", [H, H], BF16)
    W2 = din("W2", [H, H], BF16)
    wdiag = din("wdiag", [3 * RMAX, RMAX * 128], BF16)
    bmsg_row = din("bmsg_row", [1, H], BF16)
    W_res = din("W_res", [H, H], BF16)
    Wu1 = din("Wu1", [H, H], BF16)
    Wu2 = din("Wu2", [H, H], BF16)
    bupd_col = din("bupd_col", [H, 1], F32)
    iota_p = din("iota_p", [128, 1], F32)
    iota_rep = din("iota_rep", [128, 128], BF16)
    ones_bf = din("ones_bf", [128, 128], BF16)
    emb_shardT = din("emb_shardT", [H, S_pad], BF16)
    gidx16 = din("gidx16", [128, R, RMAX * 8], I16)
    colp = din("colp", [128, T], BF16)
    colf = din("colf", [R, RMAX * 128], BF16)
    distT = din("distT", [R, RMAX, 128], BF16)

    # 512B-stride rows: dma_gather hangs on 256B-stride tables; only the
    # lower 128 elems are written/gathered (elem_step=256).
    At_lo = nc.dram_tensor("At_lo", [N_half, 2 * H], BF16)
    At_hi = nc.dram_tensor("At_hi", [N_half, 2 * H], BF16)
    out_d = nc.dram_tensor("out", [H, S_pad], F32, kind="ExternalOutput")

    with tile.TileContext(nc) as tc:
        with (
            tc.tile_pool(name="const", bufs=1) as cp,
            tc.tile_pool(name="sb", bufs=2) as sb,
            tc.tile_pool(name="big", bufs=1) as bigp,
            tc.tile_pool(name="ps", bufs=2, space="PSUM") as ps,
            tc.tile_pool(name="aggp", bufs=2, space="PSUM") as aggp,
            tc.tile_pool(name="p3ps", bufs=2, space="PSUM") as p3ps,
        ):
            def cload(t, shape, dt):
                s = cp.tile(shape, dt, tag=t.name)
                nc.sync.dma_start(s[:], t[:])
                return s

            W1s = cload(W1, [H, H], BF16)
            W2s = cload(W2, [H, H], BF16)
            wdiags = cload(wdiag, [3 * RMAX, RMAX * 128], BF16)
            bmsgs = cload(bmsg_row, [1, H], BF16)
            Wress = cload(W_res, [H, H], BF16)
            Wu1s = cload(Wu1, [H, H], BF16)
            Wu2s = cload(Wu2, [H, H], BF16)
            bupds = cload(bupd_col, [H, 1], F32)
            iotaps = cload(iota_p, [128, 1], F32)
            iotars = cload(iota_rep, [128, 128], BF16)
            oness = cload(ones_bf, [128, 128], BF16)
            colps = cload(colp, [128, T], BF16)
            emb_sb = bigp.tile([H, S_pad], BF16, tag="emb_sb")
            nc.sync.dma_start(emb_sb[:], emb_shardT[:])
            Bres = bigp.tile([128, n_win, H], BF16, tag="Bres")
            aggrT = bigp.tile([128, S_pad], BF16, tag="aggrT")

            # pin the sb-pool tag layout: gather timing is sensitive to the
            # SBUF addresses of the edge-phase tiles, so reserve every tag in
            # a fixed order up front; new/bigger tags must go after these.
            for _tag, _shape, _dt, _bufs in (
                ("atstage", [128, 16, H], BF16, 2),
                ("embT4", [H, 512], BF16, 2),
                ("il", [128, 8, RMAX * 8], I16, 2),
                ("colf_g", [128, RMAX * 128], BF16, 2),
                ("dT", [RMAX, 128], BF16, 4),
                ("Ag", [128, RMAX, H], BF16, 2),
                ("oT", [128, RMAX, H], BF16, 2),
                ("o8", [128, RMAX, H], BF16, 2),
                ("msgf", [128, RMAX, H], F32, 2),
                ("msgb", [128, RMAX, H], BF16, 2),
                ("p3r", [128, 512], F32, 2),
                ("p3o", [128, 512], F32, 2),
                ("embT8", [H, 1024], BF16, 2),
            ):
                _pin = sb.tile(_shape, _dt, tag=_tag, bufs=_bufs, name="pin")

            # ---------- A tables: emb @ W1, slot-permuted bf16 -------------
            # one emitter per 16-chunk stage group; At_hi groups are emitted
            # interleaved into the first edge-loop runs
            # full 512B-row stage (upper halves zeroed) -> 8KB-contiguous
            # per-partition table writes instead of a 256B-descriptor spray
            def at_group(dst, half, q0, g, weng):
                stage = sb.tile([128, 16, 2 * H], BF16, tag="atstage2",
                                name="stage")
                nc.vector.memset(stage[:, :, H:2 * H], 0.0)
                for j0 in range(0, g, 8):
                    nj = min(8, g - j0)
                    embT8 = sb.tile([H, 1024], BF16, tag="embT8", name="embT8")
                    off = half * N_half + (q0 + j0) * 128
                    nc.sync.dma_start(embT8[:, 0:nj * 128],
                                      embT_full[:, off:off + nj * 128])
                    psA = ps.tile([128, RMAX, H], F32, tag="msgps", name="psA")
                    for j in range(nj):
                        nc.tensor.matmul(out=psA[:, j, :],
                                         lhsT=embT8[:, j * 128:(j + 1) * 128],
                                         rhs=W1s[:], start=True, stop=True)
                    nc.scalar.activation(
                        out=stage[:, j0:j0 + nj, 0:H], in_=psA[:, 0:nj, :],
                        func=mybir.ActivationFunctionType.Copy)
                weng.dma_start(
                    out=dst.rearrange("(p q) f -> p q f", p=128)
                    [:, q0:q0 + g, :],
                    in_=stage[:, 0:g, :])

            gi = 0
            for q0 in range(0, CH, 16):
                at_group(At_lo, 0, q0, min(16, CH - q0),
                         nc.scalar if gi % 2 else nc.sync)
                gi += 1
            hi_groups = [(q0, min(16, CH - q0)) for q0 in range(0, CH, 16)]
            hi_next = 0

            # ---------- Bres: emb_shard @ W2 + b, SBUF resident ------------
            # after At_lo on the PE stream: only needed once gathers flow
            for w in range(n_win):
                psB = aggp.tile([128, H], F32, tag="aggr")
                nc.tensor.matmul(out=psB[:], lhsT=emb_sb[:, w * 128:(w + 1) * 128],
                                 rhs=W2s[:], start=True, stop=False)
                nc.tensor.matmul(out=psB[:], lhsT=oness[0:1, :], rhs=bmsgs[:],
                                 start=False, stop=True)
                nc.scalar.activation(out=Bres[:, w, :], in_=psB[:],
                                     func=mybir.ActivationFunctionType.Copy)

            # ---------- node update MLP, one 512-col block -----------------
            def emit_p3_block(b0):
                nb = min(512, S_pad - b0)
                ps_u = p3ps.tile([128, 512], F32, tag="p3ps", name="ps_u")
                pu = ps_u[:]
                nc.tensor.matmul(out=pu[:, 0:nb], lhsT=Wu1s[:],
                                 rhs=emb_sb[:, b0:b0 + nb], start=True,
                                 stop=False)
                nc.tensor.matmul(out=pu[:, 0:nb], lhsT=Wu2s[:],
                                 rhs=aggrT[:, b0:b0 + nb], start=False,
                                 stop=True)
                r_sb = sb.tile([128, 512], F32, tag="p3r", name="r_sb")
                nc.scalar.activation(out=r_sb[:, 0:nb], in_=pu[:, 0:nb],
                                     func=mybir.ActivationFunctionType.Relu,
                                     bias=bupds[:])
                ps_r = p3ps.tile([128, 512], F32, tag="p3ps", name="ps_r")
                pr = ps_r[:]
                nc.tensor.matmul(out=pr[:, 0:nb], lhsT=Wress[:],
                                 rhs=emb_sb[:, b0:b0 + nb], start=True,
                                 stop=True)
                o_sb = sb.tile([128, 512], F32, tag="p3o", name="o_sb")
                nc.vector.tensor_tensor(out=o_sb[:, 0:nb], in0=r_sb[:, 0:nb],
                                        in1=pr[:, 0:nb],
                                        op=mybir.AluOpType.add)
                nc.scalar.dma_start(out_d[:, b0:b0 + nb], o_sb[:, 0:nb])

            # window w is final after its last tile in the latest phase that
            # has tiles; map final tiles -> ready P3 blocks
            fin_tile = {}
            for w in range(n_win):
                if (w, 1) in last_wp:
                    fin_tile[w] = last_wp[(w, 1)]
                elif (w, 0) in last_wp:
                    fin_tile[w] = last_wp[(w, 0)]
            blocks_after = {}
            for b0 in range(0, S_pad, 512):
                wins = [w for w in range(b0 // 128, min((b0 + 512), S_pad) // 128)]
                fins = [fin_tile[w] for w in wins if w in fin_tile]
                if fins:
                    blocks_after.setdefault(max(fins), []).append(b0)
            p3_emitted = set()

            # ---------- edge loop ------------------------------------------
            # zero windows that never receive edges, before any P3 block runs
            for w in range(n_win):
                if w not in fin_tile:
                    nc.vector.memset(aggrT[:, w * 128:(w + 1) * 128], 0.0)

            aggr_tiles = {}
            win_written = set()
            il_blk = None
            colf_blk = None

            for ri, (t0, L, hf) in enumerate(runs if not skip_edges else []):
                # drip the At_hi build into the early lo-phase runs (all of it
                # before the first hi gather)
                if hi_next < len(hi_groups) and (ri >= 1 or hf == 1):
                    n_emit = len(hi_groups) - hi_next if hf == 1 else 1
                    for _ in range(n_emit):
                        q0, g = hi_groups[hi_next]
                        at_group(At_hi, 1, q0, g, nc.scalar)
                        hi_next += 1
                if ri % 8 == 0:
                    nb = min(8, R - ri)
                    il_blk = sb.tile([128, 8, RMAX * 8], I16, tag="il")
                    nc.sync.dma_start(il_blk[:, 0:nb, :],
                                      gidx16[:, ri:ri + nb, :])
                if ri % 3 == 0:
                    nbc = min(3, R - ri)
                    colf_blk = sb.tile([128, RMAX * 128], BF16, tag="colf_g")
                    nc.sync.dma_start(colf_blk[0:nbc * 32:32, :],
                                      colf[ri:ri + nbc, :])
                il = il_blk[:, ri % 8, :]
                bp = 32 * (ri % 3)
                colf_g = colf_blk[bp:bp + 1, :]

                dT = sb.tile([RMAX, 128], BF16, tag="dT", bufs=4)
                nc.sync.dma_start(dT[0:L, :], distT[ri, 0:L, :])

                Ag = sb.tile([128, RMAX, H], BF16, tag="Ag")
                src = (At_lo if hf == 0 else At_hi)[:, 0:H]
                if fake_gather:
                    r0f = (ri * 1024) % (N_half - RMAX * 128)
                    nc.gpsimd.dma_start(
                        out=Ag[:, 0:L, :],
                        in_=(At_lo if hf == 0 else At_hi)
                        [r0f:r0f + L * 128, :]
                        .rearrange("(k p) f -> p k f", p=128))
                else:
                    nc.gpsimd.dma_gather(Ag[:, 0:L, :], src, il[:, 0:L * 8],
                                         L * 128, L * 128, H, elem_step=2 * H)

                msg_ps = ps.tile([128, RMAX, H], F32, tag="msgps")
                flat = msg_ps[:].rearrange("p k e -> p (k e)")
                # broadcast colf into all partitions (rank-1 bf16 matmuls)
                for o in range(0, L * 128, 512):
                    oe = min(o + 512, L * 128)
                    nc.tensor.matmul(out=flat[:, o:oe],
                                     lhsT=oness[bp:bp + 1, :],
                                     rhs=colf_g[:, o:oe], start=True, stop=True)
                oT = sb.tile([128, RMAX, H], BF16, tag="oT")
                nc.vector.tensor_tensor(
                    out=oT[:, 0:L, :],
                    in0=iotaps[:, :, None].to_broadcast([128, L, 128]),
                    in1=msg_ps[:, 0:L, :], op=mybir.AluOpType.is_equal)
                o8 = sb.tile([128, RMAX, H], BF16, tag="o8")
                nc.vector.tensor_tensor(
                    out=o8[:, 0:L, :],
                    in0=colps[:, t0:t0 + L, None].to_broadcast([128, L, 128]),
                    in1=iotars[:, None, :].to_broadcast([128, L, 128]),
                    op=mybir.AluOpType.is_equal)
                # message: dist*w_d (block-diag) + one-hot-gathered B
                for o in range(0, L * 128, 512):
                    oe = min(o + 512, L * 128)
                    nc.tensor.matmul(out=flat[:, o:oe], lhsT=dT[0:L, :],
                                     rhs=wdiags[0:L, o:oe], start=True,
                                     stop=False, skip_group_check=True)
                for k in range(L):
                    w = win_of_tile[t0 + k]
                    nc.tensor.matmul(out=msg_ps[:, k, :], lhsT=oT[:, k, :],
                                     rhs=Bres[:, w, :], start=False, stop=True,
                                     skip_group_check=True)
                # + gathered A (DVE), relu+cast (ACT)
                msg_f = sb.tile([128, RMAX, H], F32, tag="msgf")
                nc.vector.tensor_tensor(out=msg_f[:, 0:L, :], in0=Ag[:, 0:L, :],
                                        in1=msg_ps[:, 0:L, :],
                                        op=mybir.AluOpType.add)
                msg_bf = sb.tile([128, RMAX, H], BF16, tag="msgb")
                nc.scalar.activation(out=msg_bf[:, 0:L, :], in_=msg_f[:, 0:L, :],
                                     func=mybir.ActivationFunctionType.Relu)
                # segment-sum into per-window psum
                for k in range(L):
                    t = t0 + k
                    w = win_of_tile[t]
                    if t == first_wp[(w, hf)]:
                        aggr_t = aggp.tile([128, H], F32, tag="aggr")
                        aggr_tiles[w] = aggr_t
                    nc.tensor.matmul(out=aggr_tiles[w][:],
                                     lhsT=msg_bf[:, k, :], rhs=o8[:, k, :],
                                     start=(t == first_wp[(w, hf)]),
                                     stop=(t == last_wp[(w, hf)]))
                    if t == last_wp[(w, hf)]:
                        dstw = aggrT[:, w * 128:(w + 1) * 128]
                        if w in win_written:
                            nc.vector.tensor_tensor(
                                out=dstw, in0=dstw, in1=aggr_tiles[w][:],
                                op=mybir.AluOpType.add)
                        else:
                            nc.scalar.activation(
                                out=dstw, in_=aggr_tiles[w][:],
                                func=mybir.ActivationFunctionType.Copy)
                            win_written.add(w)
                        del aggr_tiles[w]
                        for b0 in blocks_after.get(t, []):
                            emit_p3_block(b0)
                            p3_emitted.add(b0)

            while hi_next < len(hi_groups):
                q0, g = hi_groups[hi_next]
                at_group(At_hi, 1, q0, g, nc.scalar)
                hi_next += 1
            # any blocks not emitted inside the loop (e.g. skip_edges)
            for b0 in range(0, S_pad, 512):
                if b0 not in p3_emitted:
                    emit_p3_block(b0)

    nc.compile()
    return nc


# --------------------------------------------------------------------------
# entry point
# --------------------------------------------------------------------------

def kernel(node_embed, node_pos, W_res, W_msg, b_msg, W_upd, b_upd,
           edge_index, n_cores=8, _run=None):
    cfg, in_maps = host_prep(node_embed, node_pos, W_res, W_msg, b_msg,
                             W_upd, b_upd, edge_index, n_cores)
    nc = build_program(cfg)
    if _run is None:
        res = run_bass_kernel_spmd(nc, in_maps, core_ids=list(range(n_cores)))
        outs = [res.results[c]["out"] for c in range(n_cores)]
    else:
        outs = _run(nc, in_maps)
    S = cfg["S"]
    return np.concatenate([o[:, :S].T for o in outs], axis=0)



# revision 5
# speedup vs baseline: 1.5703x; 1.5703x over previous
"""Trainium2 Bass kernel for an equivariant GNN message-passing layer.

Full inputs in, full output out. 8-way owner-computes sharding by edge target
node (col). Per-edge squared distances are precomputed on host; the device
computes, per core c (nodes [c*S, (c+1)*S)):

  A-table:  At[n]   = emb[n] @ W1            (bf16, DRAM, slot-permuted rows)
  B-window: Bres[w] = emb_shard[w] @ W2 + b  (bf16, SBUF resident)
  msg[e]    = relu(At[row_e] + Bres[col_e] + dist_e * w_d)
  aggrT     = one-hot scatter-sum of msg by col      [128, S_pad] bf16
  outT      = Wres^T emb^T + relu(Wu1^T emb^T + Wu2^T aggrT + b_upd)

with W1 = W_msg[:128], W2 = W_msg[128:256], w_d = W_msg[256].

At rows are fetched per edge with gpsimd dma_gather (256B rows, int16
indices). The table is split in two halves so slot ids fit int16; within a
half, node r maps to slot (r%128)*C + r//128 (C=196 chunks) so the table
build writes 4KB-contiguous per-partition spans. All matmuls are bf16
(fp32 matmuls double-pump on TRN2). The output is produced transposed
[128, S_pad]; the host transposes back.
"""

import sys

for _p in ("/opt/trn_rl_repo",):
    if _p not in sys.path:
        sys.path.insert(0, _p)

import numpy as np
import ml_dtypes

import concourse.bacc as bacc
import concourse.bass as bass
import concourse.mybir as mybir
import concourse.tile as tile
from concourse.bass_utils import run_bass_kernel_spmd

F32 = mybir.dt.float32
BF16 = mybir.dt.bfloat16
I16 = mybir.dt.int16
BF = ml_dtypes.bfloat16

H = 128          # hidden/in channels (hardcoded for this problem)
RMAX = 8         # max 128-edge tiles per gather run


# --------------------------------------------------------------------------
# host-side prep
# --------------------------------------------------------------------------

def host_prep(node_embed, node_pos, W_res, W_msg, b_msg, W_upd, b_upd,
              edge_index, n_cores):
    N, C_in = node_embed.shape
    assert C_in == H and W_msg.shape == (2 * H + 1, H)
    assert N % n_cores == 0
    S = N // n_cores
    n_win = -(-S // 128)
    S_pad = n_win * 128
    N_pad = -(-N // 256) * 256
    N_half = N_pad // 2
    CH = N_half // 128                  # chunks per half (196)

    row = np.asarray(edge_index[0], dtype=np.int64)
    col = np.asarray(edge_index[1], dtype=np.int64)
    pos = np.asarray(node_pos, dtype=np.float32)
    diff = pos[row] - pos[col]
    dist = np.sum(diff * diff, axis=1).astype(np.float32)   # [E]

    core_of = col // S

    per_core = []
    counts = np.zeros((n_cores, 2, n_win), dtype=np.int64)
    for c in range(n_cores):
        sel = np.nonzero(core_of == c)[0]
        lc = col[sel] - c * S
        w = lc // 128
        hf = (row[sel] >= N_half).astype(np.int64)
        order = np.lexsort((w, hf))
        sel, w, hf = sel[order], w[order], hf[order]
        cw = (lc[order] % 128).astype(np.float32)
        np.add.at(counts[c], (hf, w), 1)
        per_core.append((sel, cw))

    tiles_hw = -(-counts.max(axis=0) // 128)            # [2, n_win]
    win_of_tile, half_of_tile = [], []
    for hf in (0, 1):
        for w in range(n_win):
            win_of_tile += [w] * int(tiles_hw[hf, w])
            half_of_tile += [hf] * int(tiles_hw[hf, w])
    T = len(win_of_tile)
    first_wp, last_wp = {}, {}
    for t, (w, hf) in enumerate(zip(win_of_tile, half_of_tile)):
        first_wp.setdefault((w, hf), t)
        last_wp[(w, hf)] = t

    # gather runs: consecutive same-half tiles, capped at RMAX
    runs = []
    t = 0
    while t < T:
        hf = half_of_tile[t]
        L = 1
        while (t + L < T and half_of_tile[t + L] == hf and L < RMAX):
            L += 1
        runs.append((t, L, hf))
        t += L
    R = len(runs)

    tile_base = {}
    b = 0
    for hf in (0, 1):
        for w in range(n_win):
            tile_base[(hf, w)] = b
            b += int(tiles_hw[hf, w])

    # per-core index/side arrays
    gidx_all, colp_all, colf_all, distT_all = [], [], [], []
    for c in range(n_cores):
        sel, cw = per_core[c]
        slots16 = np.zeros((T, 128), dtype=np.int16)
        colp = np.full((128, T), -1.0, dtype=np.float32)
        distp = np.zeros((T, 128), dtype=np.float32)
        start = 0
        for hf in (0, 1):
            for w in range(n_win):
                cnt = int(counts[c, hf, w])
                if cnt:
                    idx = np.arange(cnt)
                    t_loc = tile_base[(hf, w)] + idx // 128
                    lane = idx % 128
                    r = row[sel[start:start + cnt]] - hf * N_half
                    slot = (r % 128) * CH + r // 128
                    slots16[t_loc, lane] = slot.astype(np.int16)
                    distp[t_loc, lane] = dist[sel[start:start + cnt]]
                    start += cnt
                    colp[lane, t_loc] = cw[start - cnt:start]
        colf = np.full((R, RMAX * 128), -1.0, dtype=np.float32)
        distT = np.zeros((R, RMAX, 128), dtype=np.float32)
        gidx16 = np.zeros((R, 128, RMAX * 8), dtype=np.int16)
        for ri, (t0, L, hf) in enumerate(runs):
            flat = slots16[t0:t0 + L].reshape(L * 128)
            wrap = flat.reshape(-1, 16).T
            for rep in range(8):
                gidx16[ri, rep * 16:(rep + 1) * 16, :L * 8] = wrap
            for k in range(L):
                colf[ri, k * 128:(k + 1) * 128] = colp[:, t0 + k]
                distT[ri, k] = distp[t0 + k]
        gidx_all.append(np.ascontiguousarray(gidx16.transpose(1, 0, 2)))
        colp_all.append(colp.astype(BF))
        colf_all.append(colf.astype(BF))
        distT_all.append(distT.astype(BF))

    # replicated tensors
    emb = np.asarray(node_embed, dtype=np.float32)
    embT_full = np.zeros((H, N_pad), dtype=BF)
    embT_full[:, :N] = emb.T.astype(BF)

    iota = np.arange(128, dtype=np.float32)
    W_msg = np.asarray(W_msg, dtype=np.float32)
    W_upd = np.asarray(W_upd, dtype=np.float32)
    # three replicated row-blocks (partitions 0-7/8-15/16-23) so the dist
    # lhsT can sit at any of the 3 rotating dT partition bases
    wdiag = np.zeros((3 * RMAX, RMAX * 128), dtype=np.float32)
    for j in range(3):
        for k in range(RMAX):
            wdiag[j * RMAX + k, k * 128:(k + 1) * 128] = W_msg[2 * H]
    repl = {
        "W1": np.ascontiguousarray(W_msg[:H]).astype(BF),
        "W2": np.ascontiguousarray(W_msg[H:2 * H]).astype(BF),
        "wdiag": wdiag.astype(BF),
        "bmsg_row": np.asarray(b_msg, dtype=np.float32).reshape(1, H).astype(BF),
        "W_res": np.asarray(W_res, dtype=np.float32).astype(BF),
        "Wu1": np.ascontiguousarray(W_upd[:H]).astype(BF),
        "Wu2": np.ascontiguousarray(W_upd[H:]).astype(BF),
        "bupd_col": np.asarray(b_upd, dtype=np.float32).reshape(H, 1),
        "iota_p": iota.reshape(128, 1).copy(),
        "iota_rep": np.tile(iota.reshape(1, 128), (128, 1)).astype(BF),
        "ones_bf": np.ones((128, 128), np.float32).astype(BF),
        "embT_full": embT_full,
    }

    in_maps = []
    for c in range(n_cores):
        shardT = np.zeros((H, S_pad), dtype=BF)
        shardT[:, :S] = emb[c * S:(c + 1) * S].T.astype(BF)
        m = dict(repl)
        m["emb_shardT"] = shardT
        m["gidx16"] = gidx_all[c]
        m["colp"] = colp_all[c]
        m["colf"] = colf_all[c]
        m["distT"] = distT_all[c]
        in_maps.append(m)

    cfg = dict(N=N, N_pad=N_pad, N_half=N_half, CH=CH, S=S, S_pad=S_pad,
               n_win=n_win, R=R, T=T, runs=runs, win_of_tile=win_of_tile,
               half_of_tile=half_of_tile, first_wp=first_wp, last_wp=last_wp,
               n_cores=n_cores)
    return cfg, in_maps


# --------------------------------------------------------------------------
# device program
# --------------------------------------------------------------------------

def build_program(cfg, debug=False, fake_gather=False, skip_edges=False):
    N_half, CH, S_pad, n_win, R, T = (cfg["N_half"], cfg["CH"], cfg["S_pad"],
                                      cfg["n_win"], cfg["R"], cfg["T"])
    runs = cfg["runs"]
    win_of_tile = cfg["win_of_tile"]
    first_wp, last_wp = cfg["first_wp"], cfg["last_wp"]

    nc = bacc.Bacc("TRN2", target_bir_lowering=False, debug=debug,
                   num_devices=cfg["n_cores"], num_swdge_queues=4)

    din = lambda n, s, dt: nc.dram_tensor(n, s, dt, kind="ExternalInput")
    embT_full = din("embT_full", [H, cfg["N_pad"]], BF16)
    W1 = din("W1", [H, H], BF16)
    W2 = din("W2", [H, H], BF16)
    wdiag = din("wdiag", [3 * RMAX, RMAX * 128], BF16)
    bmsg_row = din("bmsg_row", [1, H], BF16)
    W_res = din("W_res", [H, H], BF16)
    Wu1 = din("Wu1", [H, H], BF16)
    Wu2 = din("Wu2", [H, H], BF16)
    bupd_col = din("bupd_col", [H, 1], F32)
    iota_p = din("iota_p", [128, 1], F32)
    iota_rep = din("iota_rep", [128, 128], BF16)
    ones_bf = din("ones_bf", [128, 128], BF16)
    emb_shardT = din("emb_shardT", [H, S_pad], BF16)
    gidx16 = din("gidx16", [128, R, RMAX * 8], I16)
    colp = din("colp", [128, T], BF16)
    colf = din("colf", [R, RMAX * 128], BF16)
    distT = din("distT", [R, RMAX, 128], BF16)

    # 512B-stride rows: dma_gather hangs on 256B-stride tables; only the
    # lower 128 elems are written/gathered (elem_step=256).
    At_lo = nc.dram_tensor("At_lo", [N_half, 2 * H], BF16)
    At_hi = nc.dram_tensor("At_hi", [N_half, 2 * H], BF16)
    out_d = nc.dram_tensor("out", [H, S_pad], F32, kind="ExternalOutput")

    with tile.TileContext(nc) as tc:
        with (
            tc.tile_pool(name="const", bufs=1) as cp,
            tc.tile_pool(name="sb", bufs=2) as sb,
            tc.tile_pool(name="big", bufs=1) as bigp,
            tc.tile_pool(name="ps", bufs=2, space="PSUM") as ps,
            tc.tile_pool(name="aggp", bufs=2, space="PSUM") as aggp,
            tc.tile_pool(name="p3ps", bufs=2, space="PSUM") as p3ps,
        ):
            def cload(t, shape, dt):
                s = cp.tile(shape, dt, tag=t.name)
                nc.sync.dma_start(s[:], t[:])
                return s

            W1s = cload(W1, [H, H], BF16)
            W2s = cload(W2, [H, H], BF16)
            wdiags = cload(wdiag, [3 * RMAX, RMAX * 128], BF16)
            bmsgs = cload(bmsg_row, [1, H], BF16)
            Wress = cload(W_res, [H, H], BF16)
            Wu1s = cload(Wu1, [H, H], BF16)
            Wu2s = cload(Wu2, [H, H], BF16)
            bupds = cload(bupd_col, [H, 1], F32)
            iotaps = cload(iota_p, [128, 1], F32)
            iotars = cload(iota_rep, [128, 128], BF16)
            oness = cload(ones_bf, [128, 128], BF16)
            colps = cload(colp, [128, T], BF16)
            emb_sb = bigp.tile([H, S_pad], BF16, tag="emb_sb")
            nc.sync.dma_start(emb_sb[:], emb_shardT[:])
            Bres = bigp.tile([128, n_win, H], BF16, tag="Bres")
            aggrT = bigp.tile([128, S_pad], BF16, tag="aggrT")

            # pin the sb-pool tag layout: gather timing is sensitive to the
            # SBUF addresses of the edge-phase tiles, so reserve every tag in
            # a fixed order up front; new/bigger tags must go after these.
            for _tag, _shape, _dt, _bufs in (
                ("atstage", [128, 16, H], BF16, 2),
                ("embT4", [H, 512], BF16, 2),
                ("il", [128, 8, RMAX * 8], I16, 2),
                ("colf_g", [128, RMAX * 128], BF16, 2),
                ("dT", [RMAX, 128], BF16, 4),
                ("Ag", [128, RMAX, H], BF16, 4),
                ("oT", [128, RMAX, H], BF16, 2),
                ("o8", [128, RMAX, H], BF16, 2),
                ("msgf", [128, RMAX, H], F32, 2),
                ("msgb", [128, RMAX, H], BF16, 2),
                ("p3r", [128, 512], F32, 2),
                ("p3o", [128, 512], F32, 2),
                ("embT8", [H, 1024], BF16, 2),
            ):
                _pin = sb.tile(_shape, _dt, tag=_tag, bufs=_bufs, name="pin")

            # ---------- A tables: emb @ W1, slot-permuted bf16 -------------
            # one emitter per 16-chunk stage group; At_hi groups are emitted
            # interleaved into the first edge-loop runs
            # full 512B-row stage (upper halves zeroed) -> 8KB-contiguous
            # per-partition table writes instead of a 256B-descriptor spray
            def at_group(dst, half, q0, g, weng):
                stage = sb.tile([128, 16, 2 * H], BF16, tag="atstage2",
                                name="stage")
                nc.vector.memset(stage[:, :, H:2 * H], 0.0)
                for j0 in range(0, g, 8):
                    nj = min(8, g - j0)
                    embT8 = sb.tile([H, 1024], BF16, tag="embT8", name="embT8")
                    off = half * N_half + (q0 + j0) * 128
                    nc.sync.dma_start(embT8[:, 0:nj * 128],
                                      embT_full[:, off:off + nj * 128])
                    psA = ps.tile([128, RMAX, H], F32, tag="msgps", name="psA")
                    for j in range(nj):
                        nc.tensor.matmul(out=psA[:, j, :],
                                         lhsT=embT8[:, j * 128:(j + 1) * 128],
                                         rhs=W1s[:], start=True, stop=True)
                    nc.scalar.activation(
                        out=stage[:, j0:j0 + nj, 0:H], in_=psA[:, 0:nj, :],
                        func=mybir.ActivationFunctionType.Copy)
                weng.dma_start(
                    out=dst.rearrange("(p q) f -> p q f", p=128)
                    [:, q0:q0 + g, :],
                    in_=stage[:, 0:g, :])

            gi = 0
            for q0 in range(0, CH, 16):
                at_group(At_lo, 0, q0, min(16, CH - q0),
                         nc.scalar if gi % 2 else nc.sync)
                gi += 1
            hi_groups = [(q0, min(16, CH - q0)) for q0 in range(0, CH, 16)]
            hi_next = 0

            # ---------- Bres: emb_shard @ W2 + b, SBUF resident ------------
            # after At_lo on the PE stream: only needed once gathers flow
            for w in range(n_win):
                psB = aggp.tile([128, H], F32, tag="aggr")
                nc.tensor.matmul(out=psB[:], lhsT=emb_sb[:, w * 128:(w + 1) * 128],
                                 rhs=W2s[:], start=True, stop=False)
                nc.tensor.matmul(out=psB[:], lhsT=oness[0:1, :], rhs=bmsgs[:],
                                 start=False, stop=True)
                nc.scalar.activation(out=Bres[:, w, :], in_=psB[:],
                                     func=mybir.ActivationFunctionType.Copy)

            # ---------- node update MLP, one 512-col block -----------------
            def emit_p3_block(b0):
                nb = min(512, S_pad - b0)
                ps_u = p3ps.tile([128, 512], F32, tag="p3ps", name="ps_u")
                pu = ps_u[:]
                nc.tensor.matmul(out=pu[:, 0:nb], lhsT=Wu1s[:],
                                 rhs=emb_sb[:, b0:b0 + nb], start=True,
                                 stop=False)
                nc.tensor.matmul(out=pu[:, 0:nb], lhsT=Wu2s[:],
                                 rhs=aggrT[:, b0:b0 + nb], start=False,
                                 stop=True)
                r_sb = sb.tile([128, 512], F32, tag="p3r", name="r_sb")
                nc.scalar.activation(out=r_sb[:, 0:nb], in_=pu[:, 0:nb],
                                     func=mybir.ActivationFunctionType.Relu,
                                     bias=bupds[:])
                ps_r = p3ps.tile([128, 512], F32, tag="p3ps", name="ps_r")
                pr = ps_r[:]
                nc.tensor.matmul(out=pr[:, 0:nb], lhsT=Wress[:],
                                 rhs=emb_sb[:, b0:b0 + nb], start=True,
                                 stop=True)
                o_sb = sb.tile([128, 512], F32, tag="p3o", name="o_sb")
                nc.vector.tensor_tensor(out=o_sb[:, 0:nb], in0=r_sb[:, 0:nb],
                                        in1=pr[:, 0:nb],
                                        op=mybir.AluOpType.add)
                nc.scalar.dma_start(out_d[:, b0:b0 + nb], o_sb[:, 0:nb])

            # window w is final after its last tile in the latest phase that
            # has tiles; map final tiles -> ready P3 blocks
            fin_tile = {}
            for w in range(n_win):
                if (w, 1) in last_wp:
                    fin_tile[w] = last_wp[(w, 1)]
                elif (w, 0) in last_wp:
                    fin_tile[w] = last_wp[(w, 0)]
            blocks_after = {}
            for b0 in range(0, S_pad, 512):
                wins = [w for w in range(b0 // 128, min((b0 + 512), S_pad) // 128)]
                fins = [fin_tile[w] for w in wins if w in fin_tile]
                if fins:
                    blocks_after.setdefault(max(fins), []).append(b0)
            p3_emitted = set()

            # ---------- edge loop ------------------------------------------
            # zero windows that never receive edges, before any P3 block runs
            for w in range(n_win):
                if w not in fin_tile:
                    nc.vector.memset(aggrT[:, w * 128:(w + 1) * 128], 0.0)

            aggr_tiles = {}
            win_written = set()
            il_blk = None
            colf_blk = None

            for ri, (t0, L, hf) in enumerate(runs if not skip_edges else []):
                # drip the At_hi build into the early lo-phase runs (all of it
                # before the first hi gather)
                if hi_next < len(hi_groups) and (ri >= 1 or hf == 1):
                    n_emit = len(hi_groups) - hi_next if hf == 1 else 1
                    for _ in range(n_emit):
                        q0, g = hi_groups[hi_next]
                        at_group(At_hi, 1, q0, g, nc.scalar)
                        hi_next += 1
                if ri % 8 == 0:
                    nb = min(8, R - ri)
                    il_blk = sb.tile([128, 8, RMAX * 8], I16, tag="il")
                    nc.sync.dma_start(il_blk[:, 0:nb, :],
                                      gidx16[:, ri:ri + nb, :])
                if ri % 3 == 0:
                    nbc = min(3, R - ri)
                    colf_blk = sb.tile([128, RMAX * 128], BF16, tag="colf_g")
                    nc.sync.dma_start(colf_blk[0:nbc * 32:32, :],
                                      colf[ri:ri + nbc, :])
                il = il_blk[:, ri % 8, :]
                bp = 32 * (ri % 3)
                colf_g = colf_blk[bp:bp + 1, :]

                dT = sb.tile([RMAX, 128], BF16, tag="dT", bufs=4)
                nc.sync.dma_start(dT[0:L, :], distT[ri, 0:L, :])

                Ag = sb.tile([128, RMAX, H], BF16, tag="Ag", bufs=4)
                src = (At_lo if hf == 0 else At_hi)[:, 0:H]
                if fake_gather:
                    r0f = (ri * 1024) % (N_half - RMAX * 128)
                    nc.gpsimd.dma_start(
                        out=Ag[:, 0:L, :],
                        in_=(At_lo if hf == 0 else At_hi)
                        [r0f:r0f + L * 128, :]
                        .rearrange("(k p) f -> p k f", p=128))
                else:
                    nc.gpsimd.dma_gather(Ag[:, 0:L, :], src, il[:, 0:L * 8],
                                         L * 128, L * 128, H, elem_step=2 * H,
                                         queue_num=ri % 4)

                msg_ps = ps.tile([128, RMAX, H], F32, tag="msgps")
                flat = msg_ps[:].rearrange("p k e -> p (k e)")
                # broadcast colf into all partitions (rank-1 bf16 matmuls)
                for o in range(0, L * 128, 512):
                    oe = min(o + 512, L * 128)
                    nc.tensor.matmul(out=flat[:, o:oe],
                                     lhsT=oness[bp:bp + 1, :],
                                     rhs=colf_g[:, o:oe], start=True, stop=True)
                oT = sb.tile([128, RMAX, H], BF16, tag="oT")
                nc.vector.tensor_tensor(
                    out=oT[:, 0:L, :],
                    in0=iotaps[:, :, None].to_broadcast([128, L, 128]),
                    in1=msg_ps[:, 0:L, :], op=mybir.AluOpType.is_equal)
                o8 = sb.tile([128, RMAX, H], BF16, tag="o8")
                nc.vector.tensor_tensor(
                    out=o8[:, 0:L, :],
                    in0=colps[:, t0:t0 + L, None].to_broadcast([128, L, 128]),
                    in1=iotars[:, None, :].to_broadcast([128, L, 128]),
                    op=mybir.AluOpType.is_equal)
                # message: dist*w_d (block-diag) + one-hot-gathered B
                for o in range(0, L * 128, 512):
                    oe = min(o + 512, L * 128)
                    nc.tensor.matmul(out=flat[:, o:oe], lhsT=dT[0:L, :],
                                     rhs=wdiags[0:L, o:oe], start=True,
                                     stop=False, skip_group_check=True)
                for k in range(L):
                    w = win_of_tile[t0 + k]
                    nc.tensor.matmul(out=msg_ps[:, k, :], lhsT=oT[:, k, :],
                                     rhs=Bres[:, w, :], start=False, stop=True,
                                     skip_group_check=True)
                # + gathered A (DVE), relu+cast (ACT)
                msg_f = sb.tile([128, RMAX, H], F32, tag="msgf")
                nc.vector.tensor_tensor(out=msg_f[:, 0:L, :], in0=Ag[:, 0:L, :],
                                        in1=msg_ps[:, 0:L, :],
                                        op=mybir.AluOpType.add)
                msg_bf = sb.tile([128, RMAX, H], BF16, tag="msgb")
                nc.scalar.activation(out=msg_bf[:, 0:L, :], in_=msg_f[:, 0:L, :],
                                     func=mybir.ActivationFunctionType.Relu)
                # segment-sum into per-window psum
                for k in range(L):
                    t = t0 + k
                    w = win_of_tile[t]
                    if t == first_wp[(w, hf)]:
                        aggr_t = aggp.tile([128, H], F32, tag="aggr")
                        aggr_tiles[w] = aggr_t
                    nc.tensor.matmul(out=aggr_tiles[w][:],
                                     lhsT=msg_bf[:, k, :], rhs=o8[:, k, :],
                                     start=(t == first_wp[(w, hf)]),
                                     stop=(t == last_wp[(w, hf)]))
                    if t == last_wp[(w, hf)]:
                        dstw = aggrT[:, w * 128:(w + 1) * 128]
                        if w in win_written:
                            nc.vector.tensor_tensor(
                                out=dstw, in0=dstw, in1=aggr_tiles[w][:],
                                op=mybir.AluOpType.add)
                        else:
                            nc.scalar.activation(
                                out=dstw, in_=aggr_tiles[w][:],
                                func=mybir.ActivationFunctionType.Copy)
                            win_written.add(w)
                        del aggr_tiles[w]
                        for b0 in blocks_after.get(t, []):
                            emit_p3_block(b0)
                            p3_emitted.add(b0)

            while hi_next < len(hi_groups):
                q0, g = hi_groups[hi_next]
                at_group(At_hi, 1, q0, g, nc.scalar)
                hi_next += 1
            # any blocks not emitted inside the loop (e.g. skip_edges)
            for b0 in range(0, S_pad, 512):
                if b0 not in p3_emitted:
                    emit_p3_block(b0)

    nc.compile()
    return nc


# --------------------------------------------------------------------------
# entry point
# --------------------------------------------------------------------------

def kernel(node_embed, node_pos, W_res, W_msg, b_msg, W_upd, b_upd,
           edge_index, n_cores=8, _run=None):
    cfg, in_maps = host_prep(node_embed, node_pos, W_res, W_msg, b_msg,
                             W_upd, b_upd, edge_index, n_cores)
    nc = build_program(cfg)
    if _run is None:
        res = run_bass_kernel_spmd(nc, in_maps, core_ids=list(range(n_cores)))
        outs = [res.results[c]["out"] for c in range(n_cores)]
    else:
        outs = _run(nc, in_maps)
    S = cfg["S"]
    return np.concatenate([o[:, :S].T for o in outs], axis=0)



# revision 7
# speedup vs baseline: 3.3031x; 2.1035x over previous
"""Trainium2 Bass kernel for an equivariant GNN message-passing layer.

Full inputs in, full output out. 8-way owner-computes sharding by edge target
node (col). The host sorts each core's edges by target window, pads tiles to
128 lanes, and pre-gathers the raw endpoint embedding rows into sequential
slabs (srcT/dstT, transposed [ch, edge]); per-edge squared distances are
precomputed on host. The device computes, per core c (nodes [c*S, (c+1)*S)):

  msg[e]  = relu(src_e @ W1 + dst_e @ W2 + dist_e * w_d + b)   (f32 PSUM)
  aggrT   = one-hot scatter-sum of msg by col                  [128, S_pad]
  outT    = Wres^T emb^T + relu(Wu1^T emb^T + Wu2^T aggrT + b_upd)

with W1 = W_msg[:128], W2 = W_msg[128:256], w_d = W_msg[256]. All matmuls are
bf16 inputs with f32 PSUM accumulation. dist+bias enter via a single rank-9
matmul per 512 edge-columns (8 block-diagonal dist rows + a ones row streaming
[w_d blocks; b tiled]). The output is produced transposed [128, S_pad]; the
host transposes back.
"""

import sys

for _p in ("/opt/trn_rl_repo",):
    if _p not in sys.path:
        sys.path.insert(0, _p)

import numpy as np
import ml_dtypes

import concourse.bacc as bacc
import concourse.bass as bass
import concourse.mybir as mybir
import concourse.tile as tile
from concourse.bass_utils import run_bass_kernel_spmd

F32 = mybir.dt.float32
BF16 = mybir.dt.bfloat16
BF = ml_dtypes.bfloat16

H = 128          # hidden/in channels (hardcoded for this problem)
RMAX = 8         # tiles per run


# --------------------------------------------------------------------------
# host-side prep
# --------------------------------------------------------------------------

def host_prep(node_embed, node_pos, W_res, W_msg, b_msg, W_upd, b_upd,
              edge_index, n_cores):
    N, C_in = node_embed.shape
    assert C_in == H and W_msg.shape == (2 * H + 1, H)
    assert N % n_cores == 0
    S = N // n_cores
    n_win = -(-S // 128)
    S_pad = n_win * 128

    row = np.asarray(edge_index[0], dtype=np.int64)
    col = np.asarray(edge_index[1], dtype=np.int64)
    pos = np.asarray(node_pos, dtype=np.float32)
    diff = pos[row] - pos[col]
    dist = np.sum(diff * diff, axis=1).astype(np.float32)   # [E]

    core_of = col // S

    # per-core edge lists sorted by local window; global tile structure uses
    # the max count per window across cores (SPMD: one program, one shape)
    per_core = []
    counts = np.zeros((n_cores, n_win), dtype=np.int64)
    for c in range(n_cores):
        sel = np.nonzero(core_of == c)[0]
        lc = col[sel] - c * S
        w = lc // 128
        order = np.argsort(w, kind="stable")
        sel, w = sel[order], w[order]
        cw = (lc[order] % 128).astype(np.float32)
        np.add.at(counts[c], w, 1)
        per_core.append((sel, cw))

    tiles_w = -(-counts.max(axis=0) // 128)             # [n_win]
    win_of_tile = []
    for w in range(n_win):
        win_of_tile += [w] * int(tiles_w[w])
    T = len(win_of_tile)
    R = -(-T // RMAX)
    T_pad = R * RMAX
    first_wp, last_wp = {}, {}
    for t, w in enumerate(win_of_tile):
        first_wp.setdefault(w, t)
        last_wp[w] = t
    tile_base = {}
    b = 0
    for w in range(n_win):
        tile_base[w] = b
        b += int(tiles_w[w])

    embT = np.ascontiguousarray(np.asarray(node_embed, dtype=np.float32).T
                                ).astype(BF)            # [H, N]

    in_maps = []
    iota = np.arange(128, dtype=np.float32)
    W_msg = np.asarray(W_msg, dtype=np.float32)
    W_upd = np.asarray(W_upd, dtype=np.float32)
    # rank-9 dist+bias rhs: rows 0..7 block-diagonal w_d, row 8 = b tiled
    wdiag = np.zeros((RMAX + 1, RMAX * 128), dtype=np.float32)
    for k in range(RMAX):
        wdiag[k, k * 128:(k + 1) * 128] = W_msg[2 * H]
        wdiag[RMAX, k * 128:(k + 1) * 128] = np.asarray(b_msg, np.float32)
    repl = {
        "W1": np.ascontiguousarray(W_msg[:H]).astype(BF),
        "W2": np.ascontiguousarray(W_msg[H:2 * H]).astype(BF),
        "wdiag": wdiag.astype(BF),
        "W_res": np.asarray(W_res, dtype=np.float32).astype(BF),
        "Wu1": np.ascontiguousarray(W_upd[:H]).astype(BF),
        "Wu2": np.ascontiguousarray(W_upd[H:]).astype(BF),
        "bupd_col": np.asarray(b_upd, dtype=np.float32).reshape(H, 1),
        "iota_rep": np.tile(iota.reshape(1, 128), (128, 1)).astype(BF),
    }

    emb = np.asarray(node_embed, dtype=np.float32)
    for c in range(n_cores):
        sel, cw = per_core[c]
        # per-tile edge slot assignment (window-major, padded per window)
        rows_pad = np.zeros(T_pad * 128, dtype=np.int64)
        cols_pad = np.zeros(T_pad * 128, dtype=np.int64)
        valid = np.zeros(T_pad * 128, dtype=bool)
        colp = np.full((128, T_pad), -1.0, dtype=np.float32)
        distp = np.zeros((T_pad, 128), dtype=np.float32)
        start = 0
        for w in range(n_win):
            cnt = int(counts[c, w])
            if cnt:
                idx = np.arange(cnt)
                slot = (tile_base[w] + idx // 128) * 128 + idx % 128
                e = sel[start:start + cnt]
                rows_pad[slot] = row[e]
                cols_pad[slot] = col[e]
                valid[slot] = True
                distp.reshape(-1)[slot] = dist[e]
                colp[idx % 128, tile_base[w] + idx // 128] = cw[start:start + cnt]
                start += cnt
        # pre-gathered transposed slabs [ch, T_pad*128]
        srcT = embT[:, rows_pad].copy()
        dstT = embT[:, cols_pad].copy()
        srcT[:, ~valid] = 0
        dstT[:, ~valid] = 0
        # distR: per run [9, 128]: rows 0..7 = tile dists, row 8 = ones
        distR = np.zeros((RMAX + 1, R * 128), dtype=np.float32)
        dr = distp.reshape(R, RMAX, 128)
        for k in range(RMAX):
            distR[k] = dr[:, k, :].reshape(R * 128)
        distR[RMAX] = 1.0
        m = dict(repl)
        shardT = np.zeros((H, S_pad), dtype=BF)
        shardT[:, :S] = emb[c * S:(c + 1) * S].T.astype(BF)
        m["emb_shardT"] = shardT
        m["srcT"] = srcT
        m["dstT"] = dstT
        m["distR"] = distR.astype(BF)
        m["colp"] = colp.astype(BF)
        in_maps.append(m)

    cfg = dict(N=N, S=S, S_pad=S_pad, n_win=n_win, R=R, T=T, T_pad=T_pad,
               win_of_tile=win_of_tile, first_wp=first_wp, last_wp=last_wp,
               n_cores=n_cores)
    return cfg, in_maps


# --------------------------------------------------------------------------
# device program
# --------------------------------------------------------------------------

def build_program(cfg, debug=False):
    S_pad, n_win, R, T, T_pad = (cfg["S_pad"], cfg["n_win"], cfg["R"],
                                 cfg["T"], cfg["T_pad"])
    win_of_tile = cfg["win_of_tile"]
    first_wp, last_wp = cfg["first_wp"], cfg["last_wp"]

    nc = bacc.Bacc("TRN2", target_bir_lowering=False, debug=debug,
                   num_devices=cfg["n_cores"])

    din = lambda n, s, dt: nc.dram_tensor(n, s, dt, kind="ExternalInput")
    W1 = din("W1", [H, H], BF16)
    W2 = din("W2", [H, H], BF16)
    wdiag = din("wdiag", [RMAX + 1, RMAX * 128], BF16)
    W_res = din("W_res", [H, H], BF16)
    Wu1 = din("Wu1", [H, H], BF16)
    Wu2 = din("Wu2", [H, H], BF16)
    bupd_col = din("bupd_col", [H, 1], F32)
    iota_rep = din("iota_rep", [128, 128], BF16)
    emb_shardT = din("emb_shardT", [H, S_pad], BF16)
    srcT = din("srcT", [H, T_pad * 128], BF16)
    dstT = din("dstT", [H, T_pad * 128], BF16)
    distR = din("distR", [RMAX + 1, R * 128], BF16)
    colp = din("colp", [128, T_pad], BF16)

    out_d = nc.dram_tensor("out", [H, S_pad], F32, kind="ExternalOutput")

    with tile.TileContext(nc) as tc:
        with (
            tc.tile_pool(name="const", bufs=1) as cp,
            tc.tile_pool(name="sb", bufs=2) as sb,
            tc.tile_pool(name="big", bufs=1) as bigp,
            tc.tile_pool(name="ps", bufs=2, space="PSUM") as ps,
            tc.tile_pool(name="aggp", bufs=2, space="PSUM") as aggp,
            tc.tile_pool(name="p3ps", bufs=2, space="PSUM") as p3ps,
        ):
            def cload(t, shape, dt):
                s = cp.tile(shape, dt, tag=t.name)
                nc.sync.dma_start(s[:], t[:])
                return s

            W1s = cload(W1, [H, H], BF16)
            W2s = cload(W2, [H, H], BF16)
            wdiags = cload(wdiag, [RMAX + 1, RMAX * 128], BF16)
            Wress = cload(W_res, [H, H], BF16)
            Wu1s = cload(Wu1, [H, H], BF16)
            Wu2s = cload(Wu2, [H, H], BF16)
            bupds = cload(bupd_col, [H, 1], F32)
            iotars = cload(iota_rep, [128, 128], BF16)
            colps = cload(colp, [128, T_pad], BF16)
            distRs = cload(distR, [RMAX + 1, R * 128], BF16)
            emb_sb = bigp.tile([H, S_pad], BF16, tag="emb_sb")
            nc.sync.dma_start(emb_sb[:], emb_shardT[:])
            aggrT = bigp.tile([128, S_pad], BF16, tag="aggrT")

            # ---------- node update MLP, one 512-col block -----------------
            def emit_p3_block(b0):
                nb = min(512, S_pad - b0)
                ps_u = p3ps.tile([128, 512], F32, tag="p3ps", name="ps_u")
                pu = ps_u[:]
                nc.tensor.matmul(out=pu[:, 0:nb], lhsT=Wu1s[:],
                                 rhs=emb_sb[:, b0:b0 + nb], start=True,
                                 stop=False)
                nc.tensor.matmul(out=pu[:, 0:nb], lhsT=Wu2s[:],
                                 rhs=aggrT[:, b0:b0 + nb], start=False,
                                 stop=True)
                r_sb = sb.tile([128, 512], F32, tag="p3r", name="r_sb")
                nc.scalar.activation(out=r_sb[:, 0:nb], in_=pu[:, 0:nb],
                                     func=mybir.ActivationFunctionType.Relu,
                                     bias=bupds[:])
                ps_r = p3ps.tile([128, 512], F32, tag="p3ps", name="ps_r")
                pr = ps_r[:]
                nc.tensor.matmul(out=pr[:, 0:nb], lhsT=Wress[:],
                                 rhs=emb_sb[:, b0:b0 + nb], start=True,
                                 stop=True)
                o_sb = sb.tile([128, 512], F32, tag="p3o", name="o_sb")
                nc.vector.tensor_tensor(out=o_sb[:, 0:nb], in0=r_sb[:, 0:nb],
                                        in1=pr[:, 0:nb],
                                        op=mybir.AluOpType.add)
                nc.scalar.dma_start(out_d[:, b0:b0 + nb], o_sb[:, 0:nb])

            # window w's aggregate is final after its last tile; map final
            # tiles -> ready P3 blocks
            blocks_after = {}
            for b0 in range(0, S_pad, 512):
                wins = range(b0 // 128, min(b0 + 512, S_pad) // 128)
                fins = [last_wp[w] for w in wins if w in last_wp]
                if fins:
                    blocks_after.setdefault(max(fins), []).append(b0)
            p3_emitted = set()

            # zero windows that never receive edges, before any P3 block runs
            for w in range(n_win):
                if w not in first_wp:
                    nc.vector.memset(aggrT[:, w * 128:(w + 1) * 128], 0.0)

            # ---------- edge loop ------------------------------------------
            aggr_tiles = {}
            win_written = set()

            for ri in range(R):
                t0 = ri * RMAX
                L = min(RMAX, T - t0)

                src_sb = sb.tile([128, RMAX, H], BF16, tag="src", bufs=3)
                nc.sync.dma_start(src_sb[:, 0:L, :],
                                  srcT[:, t0 * 128:(t0 + L) * 128]
                                  .rearrange("p (k e) -> p k e", k=L))
                dst_sb = sb.tile([128, RMAX, H], BF16, tag="dst", bufs=3)
                nc.scalar.dma_start(dst_sb[:, 0:L, :],
                                    dstT[:, t0 * 128:(t0 + L) * 128]
                                    .rearrange("p (k e) -> p k e", k=L))

                msg_ps = ps.tile([128, RMAX, H], F32, tag="msgps")
                flat = msg_ps[:].rearrange("p k e -> p (k e)")
                # dist*w_d + b (rank-9, block-diagonal), opens accumulation
                for o in range(0, L * 128, 512):
                    oe = min(o + 512, L * 128)
                    nc.tensor.matmul(out=flat[:, o:oe],
                                     lhsT=distRs[:, ri * 128:(ri + 1) * 128],
                                     rhs=wdiags[:, o:oe], start=True,
                                     stop=False, skip_group_check=True)
                # + src @ W1 + dst @ W2 per tile
                for k in range(L):
                    nc.tensor.matmul(out=msg_ps[:, k, :],
                                     lhsT=src_sb[:, k, :], rhs=W1s[:],
                                     start=False, stop=False,
                                     skip_group_check=True)
                    nc.tensor.matmul(out=msg_ps[:, k, :],
                                     lhsT=dst_sb[:, k, :], rhs=W2s[:],
                                     start=False, stop=True,
                                     skip_group_check=True)
                # relu + cast (ACT)
                msg_bf = sb.tile([128, RMAX, H], BF16, tag="msgb", bufs=3)
                nc.scalar.activation(out=msg_bf[:, 0:L, :],
                                     in_=msg_ps[:, 0:L, :],
                                     func=mybir.ActivationFunctionType.Relu)
                # scatter one-hot by local col
                o8 = sb.tile([128, RMAX, H], BF16, tag="o8", bufs=3)
                nc.vector.tensor_tensor(
                    out=o8[:, 0:L, :],
                    in0=colps[:, t0:t0 + L, None].to_broadcast([128, L, 128]),
                    in1=iotars[:, None, :].to_broadcast([128, L, 128]),
                    op=mybir.AluOpType.is_equal)
                # segment-sum into per-window psum
                for k in range(L):
                    t = t0 + k
                    w = win_of_tile[t]
                    if t == first_wp[w]:
                        aggr_t = aggp.tile([128, H], F32, tag="aggr")
                        aggr_tiles[w] = aggr_t
                    nc.tensor.matmul(out=aggr_tiles[w][:],
                                     lhsT=msg_bf[:, k, :], rhs=o8[:, k, :],
                                     start=(t == first_wp[w]),
                                     stop=(t == last_wp[w]))
                    if t == last_wp[w]:
                        dstw = aggrT[:, w * 128:(w + 1) * 128]
                        nc.scalar.activation(
                            out=dstw, in_=aggr_tiles[w][:],
                            func=mybir.ActivationFunctionType.Copy)
                        del aggr_tiles[w]
                        for b0 in blocks_after.get(t, []):
                            emit_p3_block(b0)
                            p3_emitted.add(b0)

            for b0 in range(0, S_pad, 512):
                if b0 not in p3_emitted:
                    emit_p3_block(b0)

    nc.compile()
    return nc


# --------------------------------------------------------------------------
# entry point
# --------------------------------------------------------------------------

def kernel(node_embed, node_pos, W_res, W_msg, b_msg, W_upd, b_upd,
           edge_index, n_cores=8, _run=None):
    cfg, in_maps = host_prep(node_embed, node_pos, W_res, W_msg, b_msg,
                             W_upd, b_upd, edge_index, n_cores)
    nc = build_program(cfg)
    if _run is None:
        res = run_bass_kernel_spmd(nc, in_maps, core_ids=list(range(n_cores)))
        outs = [res.results[c]["out"] for c in range(n_cores)]
    else:
        outs = _run(nc, in_maps)
    S = cfg["S"]
    return np.concatenate([o[:, :S].T for o in outs], axis=0)


# revision 9
# speedup vs baseline: 3.4759x; 1.0523x over previous
"""Trainium2 Bass kernel for an equivariant GNN message-passing layer.

Full inputs in, full output out. 8-way owner-computes sharding by edge target
node (col). The host sorts each core's edges by target window, pads tiles to
128 lanes, and pre-gathers the raw endpoint embedding rows into sequential
slabs (srcT/dstT, transposed [ch, edge]); per-edge squared distances are
precomputed on host. The device computes, per core c (nodes [c*S, (c+1)*S)):

  msg[e]  = relu(src_e @ W1 + dst_e @ W2 + dist_e * w_d + b)   (f32 PSUM)
  aggrT   = one-hot scatter-sum of msg by col                  [128, S_pad]
  outT    = Wres^T emb^T + relu(Wu1^T emb^T + Wu2^T aggrT + b_upd)

with W1 = W_msg[:128], W2 = W_msg[128:256], w_d = W_msg[256]. All matmuls are
bf16 inputs with f32 PSUM accumulation. dist+bias enter via a single rank-9
matmul per 512 edge-columns (8 block-diagonal dist rows + a ones row streaming
[w_d blocks; b tiled]). The output is produced transposed [128, S_pad]; the
host transposes back.
"""

import sys

for _p in ("/opt/trn_rl_repo",):
    if _p not in sys.path:
        sys.path.insert(0, _p)

import numpy as np
import ml_dtypes

import concourse.bacc as bacc
import concourse.bass as bass
import concourse.mybir as mybir
import concourse.tile as tile
from concourse.bass_utils import run_bass_kernel_spmd

F32 = mybir.dt.float32
BF16 = mybir.dt.bfloat16
BF = ml_dtypes.bfloat16

H = 128          # hidden/in channels (hardcoded for this problem)
RMAX = 8         # tiles per run


# --------------------------------------------------------------------------
# host-side prep
# --------------------------------------------------------------------------

def host_prep(node_embed, node_pos, W_res, W_msg, b_msg, W_upd, b_upd,
              edge_index, n_cores):
    N, C_in = node_embed.shape
    assert C_in == H and W_msg.shape == (2 * H + 1, H)
    assert N % n_cores == 0
    S = N // n_cores
    n_win = -(-S // 128)
    S_pad = n_win * 128

    row = np.asarray(edge_index[0], dtype=np.int64)
    col = np.asarray(edge_index[1], dtype=np.int64)
    pos = np.asarray(node_pos, dtype=np.float32)
    diff = pos[row] - pos[col]
    dist = np.sum(diff * diff, axis=1).astype(np.float32)   # [E]

    core_of = col // S

    # per-core edge lists sorted by local window; global tile structure uses
    # the max count per window across cores (SPMD: one program, one shape)
    per_core = []
    counts = np.zeros((n_cores, n_win), dtype=np.int64)
    for c in range(n_cores):
        sel = np.nonzero(core_of == c)[0]
        lc = col[sel] - c * S
        w = lc // 128
        order = np.argsort(w, kind="stable")
        sel, w = sel[order], w[order]
        cw = (lc[order] % 128).astype(np.float32)
        np.add.at(counts[c], w, 1)
        per_core.append((sel, cw))

    tiles_w = -(-counts.max(axis=0) // 128)             # [n_win]
    win_of_tile = []
    for w in range(n_win):
        win_of_tile += [w] * int(tiles_w[w])
    T = len(win_of_tile)
    R = -(-T // RMAX)
    T_pad = R * RMAX
    first_wp, last_wp = {}, {}
    for t, w in enumerate(win_of_tile):
        first_wp.setdefault(w, t)
        last_wp[w] = t
    tile_base = {}
    b = 0
    for w in range(n_win):
        tile_base[w] = b
        b += int(tiles_w[w])

    embT = np.ascontiguousarray(np.asarray(node_embed, dtype=np.float32).T
                                ).astype(BF)            # [H, N]

    in_maps = []
    iota = np.arange(128, dtype=np.float32)
    W_msg = np.asarray(W_msg, dtype=np.float32)
    W_upd = np.asarray(W_upd, dtype=np.float32)
    # rank-9 dist+bias rhs: rows 0..7 block-diagonal w_d, row 8 = b tiled
    wdiag = np.zeros((RMAX + 1, RMAX * 128), dtype=np.float32)
    for k in range(RMAX):
        wdiag[k, k * 128:(k + 1) * 128] = W_msg[2 * H]
        wdiag[RMAX, k * 128:(k + 1) * 128] = np.asarray(b_msg, np.float32)
    repl = {
        "W1": np.ascontiguousarray(W_msg[:H]).astype(BF),
        "W2": np.ascontiguousarray(W_msg[H:2 * H]).astype(BF),
        "wdiag": wdiag.astype(BF),
        "W_res": np.asarray(W_res, dtype=np.float32).astype(BF),
        "Wu1": np.ascontiguousarray(W_upd[:H]).astype(BF),
        "Wu2": np.ascontiguousarray(W_upd[H:]).astype(BF),
        "bupd_col": np.asarray(b_upd, dtype=np.float32).reshape(H, 1),
        "iota_rep": np.tile(iota.reshape(1, 128), (128, 1)).astype(BF),
    }

    emb = np.asarray(node_embed, dtype=np.float32)
    for c in range(n_cores):
        sel, cw = per_core[c]
        # per-tile edge slot assignment (window-major, padded per window)
        rows_pad = np.zeros(T_pad * 128, dtype=np.int64)
        cols_pad = np.zeros(T_pad * 128, dtype=np.int64)
        valid = np.zeros(T_pad * 128, dtype=bool)
        colp = np.full((128, T_pad), -1.0, dtype=np.float32)
        distp = np.zeros((T_pad, 128), dtype=np.float32)
        start = 0
        for w in range(n_win):
            cnt = int(counts[c, w])
            if cnt:
                idx = np.arange(cnt)
                slot = (tile_base[w] + idx // 128) * 128 + idx % 128
                e = sel[start:start + cnt]
                rows_pad[slot] = row[e]
                cols_pad[slot] = col[e]
                valid[slot] = True
                distp.reshape(-1)[slot] = dist[e]
                colp[idx % 128, tile_base[w] + idx // 128] = cw[start:start + cnt]
                start += cnt
        # pre-gathered transposed slabs [ch, T_pad*128]
        srcT = embT[:, rows_pad].copy()
        dstT = embT[:, cols_pad].copy()
        srcT[:, ~valid] = 0
        dstT[:, ~valid] = 0
        # distR: per run [9, 128]: rows 0..7 = tile dists, row 8 = ones
        distR = np.zeros((RMAX + 1, R * 128), dtype=np.float32)
        dr = distp.reshape(R, RMAX, 128)
        for k in range(RMAX):
            distR[k] = dr[:, k, :].reshape(R * 128)
        distR[RMAX] = 1.0
        m = dict(repl)
        shardT = np.zeros((H, S_pad), dtype=BF)
        shardT[:, :S] = emb[c * S:(c + 1) * S].T.astype(BF)
        m["emb_shardT"] = shardT
        m["srcT"] = srcT
        m["dstT"] = dstT
        m["distR"] = distR.astype(BF)
        m["colp"] = colp.astype(BF)
        in_maps.append(m)

    cfg = dict(N=N, S=S, S_pad=S_pad, n_win=n_win, R=R, T=T, T_pad=T_pad,
               win_of_tile=win_of_tile, first_wp=first_wp, last_wp=last_wp,
               n_cores=n_cores)
    return cfg, in_maps


# --------------------------------------------------------------------------
# device program
# --------------------------------------------------------------------------

def build_program(cfg, debug=False):
    S_pad, n_win, R, T, T_pad = (cfg["S_pad"], cfg["n_win"], cfg["R"],
                                 cfg["T"], cfg["T_pad"])
    win_of_tile = cfg["win_of_tile"]
    first_wp, last_wp = cfg["first_wp"], cfg["last_wp"]

    nc = bacc.Bacc("TRN2", target_bir_lowering=False, debug=debug,
                   num_devices=cfg["n_cores"])

    din = lambda n, s, dt: nc.dram_tensor(n, s, dt, kind="ExternalInput")
    W1 = din("W1", [H, H], BF16)
    W2 = din("W2", [H, H], BF16)
    wdiag = din("wdiag", [RMAX + 1, RMAX * 128], BF16)
    W_res = din("W_res", [H, H], BF16)
    Wu1 = din("Wu1", [H, H], BF16)
    Wu2 = din("Wu2", [H, H], BF16)
    bupd_col = din("bupd_col", [H, 1], F32)
    iota_rep = din("iota_rep", [128, 128], BF16)
    emb_shardT = din("emb_shardT", [H, S_pad], BF16)
    srcT = din("srcT", [H, T_pad * 128], BF16)
    dstT = din("dstT", [H, T_pad * 128], BF16)
    distR = din("distR", [RMAX + 1, R * 128], BF16)
    colp = din("colp", [128, T_pad], BF16)

    out_d = nc.dram_tensor("out", [H, S_pad], F32, kind="ExternalOutput")

    with tile.TileContext(nc) as tc:
        with (
            tc.tile_pool(name="const", bufs=1) as cp,
            tc.tile_pool(name="sb", bufs=2) as sb,
            tc.tile_pool(name="big", bufs=1) as bigp,
            tc.tile_pool(name="ps", bufs=2, space="PSUM") as ps,
            tc.tile_pool(name="aggp", bufs=2, space="PSUM") as aggp,
            tc.tile_pool(name="p3ps", bufs=2, space="PSUM") as p3ps,
        ):
            def cload(t, shape, dt):
                s = cp.tile(shape, dt, tag=t.name)
                nc.sync.dma_start(s[:], t[:])
                return s

            W1s = cload(W1, [H, H], BF16)
            W2s = cload(W2, [H, H], BF16)
            wdiags = cload(wdiag, [RMAX + 1, RMAX * 128], BF16)
            Wress = cload(W_res, [H, H], BF16)
            Wu1s = cload(Wu1, [H, H], BF16)
            Wu2s = cload(Wu2, [H, H], BF16)
            bupds = cload(bupd_col, [H, 1], F32)
            iotars = cload(iota_rep, [128, 128], BF16)
            colps = cload(colp, [128, T_pad], BF16)
            distRs = cload(distR, [RMAX + 1, R * 128], BF16)
            emb_sb = bigp.tile([H, S_pad], BF16, tag="emb_sb")
            nc.sync.dma_start(emb_sb[:], emb_shardT[:])
            aggrT = bigp.tile([128, S_pad], BF16, tag="aggrT")

            # ---------- node update MLP, one 512-col block -----------------
            def emit_p3_block(b0):
                nb = min(512, S_pad - b0)
                ps_u = p3ps.tile([128, 512], F32, tag="p3ps", name="ps_u")
                pu = ps_u[:]
                nc.tensor.matmul(out=pu[:, 0:nb], lhsT=Wu1s[:],
                                 rhs=emb_sb[:, b0:b0 + nb], start=True,
                                 stop=False)
                nc.tensor.matmul(out=pu[:, 0:nb], lhsT=Wu2s[:],
                                 rhs=aggrT[:, b0:b0 + nb], start=False,
                                 stop=True)
                r_sb = sb.tile([128, 512], F32, tag="p3r", name="r_sb")
                nc.scalar.activation(out=r_sb[:, 0:nb], in_=pu[:, 0:nb],
                                     func=mybir.ActivationFunctionType.Relu,
                                     bias=bupds[:])
                ps_r = p3ps.tile([128, 512], F32, tag="p3ps", name="ps_r")
                pr = ps_r[:]
                nc.tensor.matmul(out=pr[:, 0:nb], lhsT=Wress[:],
                                 rhs=emb_sb[:, b0:b0 + nb], start=True,
                                 stop=True)
                o_sb = sb.tile([128, 512], F32, tag="p3o", name="o_sb")
                nc.vector.tensor_tensor(out=o_sb[:, 0:nb], in0=r_sb[:, 0:nb],
                                        in1=pr[:, 0:nb],
                                        op=mybir.AluOpType.add)
                nc.scalar.dma_start(out_d[:, b0:b0 + nb], o_sb[:, 0:nb])

            # window w's aggregate is final after its last tile; map final
            # tiles -> ready P3 blocks
            blocks_after = {}
            for b0 in range(0, S_pad, 512):
                wins = range(b0 // 128, min(b0 + 512, S_pad) // 128)
                fins = [last_wp[w] for w in wins if w in last_wp]
                if fins:
                    blocks_after.setdefault(max(fins), []).append(b0)
            p3_emitted = set()

            # zero windows that never receive edges, before any P3 block runs
            for w in range(n_win):
                if w not in first_wp:
                    nc.vector.memset(aggrT[:, w * 128:(w + 1) * 128], 0.0)

            # ---------- edge loop ------------------------------------------
            # segsum for run ri is emitted after run ri+1's projection
            # matmuls so the PE never stalls on relu (ACT) / o8 (DVE)
            aggr_tiles = {}
            copy_alt = [0]

            def emit_segsum(t0, L, msg_bf, o8):
                for k in range(L):
                    t = t0 + k
                    w = win_of_tile[t]
                    if t == first_wp[w]:
                        aggr_t = aggp.tile([128, H], F32, tag="aggr")
                        aggr_tiles[w] = aggr_t
                    nc.tensor.matmul(out=aggr_tiles[w][:],
                                     lhsT=msg_bf[:, k, :], rhs=o8[:, k, :],
                                     start=(t == first_wp[w]),
                                     stop=(t == last_wp[w]))
                    if t == last_wp[w]:
                        dstw = aggrT[:, w * 128:(w + 1) * 128]
                        if copy_alt[0] % 2:
                            nc.scalar.activation(
                                out=dstw, in_=aggr_tiles[w][:],
                                func=mybir.ActivationFunctionType.Copy)
                        else:
                            nc.vector.tensor_scalar_add(
                                out=dstw, in0=aggr_tiles[w][:], scalar1=0.0)
                        copy_alt[0] += 1
                        del aggr_tiles[w]
                        for b0 in blocks_after.get(t, []):
                            emit_p3_block(b0)
                            p3_emitted.add(b0)

            pending = None
            for ri in range(R):
                t0 = ri * RMAX
                L = min(RMAX, T - t0)

                src_sb = sb.tile([128, RMAX, H], BF16, tag="src", bufs=3)
                nc.sync.dma_start(src_sb[:, 0:L, :],
                                  srcT[:, t0 * 128:(t0 + L) * 128]
                                  .rearrange("p (k e) -> p k e", k=L))
                dst_sb = sb.tile([128, RMAX, H], BF16, tag="dst", bufs=3)
                nc.gpsimd.dma_start(dst_sb[:, 0:L, :],
                                    dstT[:, t0 * 128:(t0 + L) * 128]
                                    .rearrange("p (k e) -> p k e", k=L))

                msg_ps = ps.tile([128, RMAX, H], F32, tag="msgps")
                flat = msg_ps[:].rearrange("p k e -> p (k e)")
                # dist*w_d + b (rank-9, block-diagonal), opens accumulation
                for o in range(0, L * 128, 512):
                    oe = min(o + 512, L * 128)
                    nc.tensor.matmul(out=flat[:, o:oe],
                                     lhsT=distRs[:, ri * 128:(ri + 1) * 128],
                                     rhs=wdiags[:, o:oe], start=True,
                                     stop=False, skip_group_check=True)
                # + src @ W1 + dst @ W2 per tile
                for k in range(L):
                    nc.tensor.matmul(out=msg_ps[:, k, :],
                                     lhsT=src_sb[:, k, :], rhs=W1s[:],
                                     start=False, stop=False,
                                     skip_group_check=True)
                    nc.tensor.matmul(out=msg_ps[:, k, :],
                                     lhsT=dst_sb[:, k, :], rhs=W2s[:],
                                     start=False, stop=True,
                                     skip_group_check=True)
                # previous run's segment-sum (PE waits on its relu no more)
                if pending is not None:
                    emit_segsum(*pending)
                # relu + cast (ACT)
                msg_bf = sb.tile([128, RMAX, H], BF16, tag="msgb", bufs=3)
                nc.scalar.activation(out=msg_bf[:, 0:L, :],
                                     in_=msg_ps[:, 0:L, :],
                                     func=mybir.ActivationFunctionType.Relu)
                # scatter one-hot by local col
                o8 = sb.tile([128, RMAX, H], BF16, tag="o8", bufs=3)
                nc.vector.tensor_tensor(
                    out=o8[:, 0:L, :],
                    in0=colps[:, t0:t0 + L, None].to_broadcast([128, L, 128]),
                    in1=iotars[:, None, :].to_broadcast([128, L, 128]),
                    op=mybir.AluOpType.is_equal)
                pending = (t0, L, msg_bf, o8)

            if pending is not None:
                emit_segsum(*pending)
            for b0 in range(0, S_pad, 512):
                if b0 not in p3_emitted:
                    emit_p3_block(b0)

    nc.compile()
    return nc


# --------------------------------------------------------------------------
# entry point
# --------------------------------------------------------------------------

def kernel(node_embed, node_pos, W_res, W_msg, b_msg, W_upd, b_upd,
           edge_index, n_cores=8, _run=None):
    cfg, in_maps = host_prep(node_embed, node_pos, W_res, W_msg, b_msg,
                             W_upd, b_upd, edge_index, n_cores)
    nc = build_program(cfg)
    if _run is None:
        res = run_bass_kernel_spmd(nc, in_maps, core_ids=list(range(n_cores)))
        outs = [res.results[c]["out"] for c in range(n_cores)]
    else:
        outs = _run(nc, in_maps)
    S = cfg["S"]
    return np.concatenate([o[:, :S].T for o in outs], axis=0)


# revision 15
# speedup vs baseline: 3.7187x; 1.0698x over previous
"""Trainium2 Bass kernel for an equivariant GNN message-passing layer.

Full inputs in, full output out. 8-way owner-computes sharding by edge target
node (col). The host sorts each core's edges by target window, pads tiles to
128 lanes, and pre-gathers the raw endpoint embedding rows into sequential
slabs (srcT/dstT, transposed [ch, edge]); per-edge squared distances are
precomputed on host. The device computes, per core c (nodes [c*S, (c+1)*S)):

  msg[e]  = relu(src_e @ W1 + dst_e @ W2 + dist_e * w_d + b)   (f32 PSUM)
  aggrT   = one-hot scatter-sum of msg by col                  [128, S_pad]
  outT    = Wres^T emb^T + relu(Wu1^T emb^T + Wu2^T aggrT + b_upd)

with W1 = W_msg[:128], W2 = W_msg[128:256], w_d = W_msg[256]. All matmuls are
bf16 inputs with f32 PSUM accumulation. dist+bias enter via a single rank-9
matmul per 512 edge-columns (8 block-diagonal dist rows + a ones row streaming
[w_d blocks; b tiled]). The output is produced transposed [128, S_pad]; the
host transposes back.
"""

import sys

for _p in ("/opt/trn_rl_repo",):
    if _p not in sys.path:
        sys.path.insert(0, _p)

import numpy as np
import ml_dtypes

import concourse.bacc as bacc
import concourse.bass as bass
import concourse.mybir as mybir
import concourse.tile as tile
from concourse.bass_utils import run_bass_kernel_spmd

F32 = mybir.dt.float32
BF16 = mybir.dt.bfloat16
BF = ml_dtypes.bfloat16

H = 128          # hidden/in channels (hardcoded for this problem)
RMAX = 8         # tiles per run


# --------------------------------------------------------------------------
# host-side prep
# --------------------------------------------------------------------------

def host_prep(node_embed, node_pos, W_res, W_msg, b_msg, W_upd, b_upd,
              edge_index, n_cores):
    N, C_in = node_embed.shape
    assert C_in == H and W_msg.shape == (2 * H + 1, H)
    assert N % n_cores == 0
    S = N // n_cores
    n_win = -(-S // 128)
    S_pad = n_win * 128

    row = np.asarray(edge_index[0], dtype=np.int64)
    col = np.asarray(edge_index[1], dtype=np.int64)
    pos = np.asarray(node_pos, dtype=np.float32)
    diff = pos[row] - pos[col]
    dist = np.sum(diff * diff, axis=1).astype(np.float32)   # [E]

    # global 128-node blocks, assigned to (core, slot) by sorted edge count
    # round-robin so the per-slot max across cores (which sets the padded
    # tile count) tracks the mean instead of the tail
    NB = -(-N // 128)
    assert n_cores * n_win >= NB
    blk_of_edge = col // 128
    cnt_g = np.bincount(blk_of_edge, minlength=NB)
    order = np.argsort(-cnt_g, kind="stable")
    blk_at = np.full((n_cores, n_win), -1, dtype=np.int64)  # (c, slot) -> blk
    core_of_blk = np.zeros(NB, dtype=np.int64)
    slot_of_blk = np.zeros(NB, dtype=np.int64)
    for j in range(n_win):
        for c in range(n_cores):
            i = j * n_cores + c
            if i < NB:
                g = order[i]
                blk_at[c, j] = g
                core_of_blk[g] = c
                slot_of_blk[g] = j

    core_of = core_of_blk[blk_of_edge]

    # per-core edge lists sorted by slot
    per_core = []
    counts = np.zeros((n_cores, n_win), dtype=np.int64)
    for c in range(n_cores):
        sel = np.nonzero(core_of == c)[0]
        w = slot_of_blk[blk_of_edge[sel]]
        order_e = np.argsort(w, kind="stable")
        sel, w = sel[order_e], w[order_e]
        cw = (col[sel] % 128).astype(np.float32)
        np.add.at(counts[c], w, 1)
        per_core.append((sel, cw))

    tiles_w = -(-counts.max(axis=0) // 128)             # [n_win]
    win_of_tile = []
    for w in range(n_win):
        win_of_tile += [w] * int(tiles_w[w])
    T = len(win_of_tile)
    R = -(-T // RMAX)
    T_pad = R * RMAX
    first_wp, last_wp = {}, {}
    for t, w in enumerate(win_of_tile):
        first_wp.setdefault(w, t)
        last_wp[w] = t
    tile_base = {}
    b = 0
    for w in range(n_win):
        tile_base[w] = b
        b += int(tiles_w[w])

    embT = np.ascontiguousarray(np.asarray(node_embed, dtype=np.float32).T
                                ).astype(BF)            # [H, N]

    in_maps = []
    iota = np.arange(128, dtype=np.float32)
    W_msg = np.asarray(W_msg, dtype=np.float32)
    W_upd = np.asarray(W_upd, dtype=np.float32)
    # rank-9 dist+bias rhs: rows 0..7 block-diagonal w_d, row 8 = b tiled
    wdiag = np.zeros((RMAX + 1, RMAX * 128), dtype=np.float32)
    for k in range(RMAX):
        wdiag[k, k * 128:(k + 1) * 128] = W_msg[2 * H]
        wdiag[RMAX, k * 128:(k + 1) * 128] = np.asarray(b_msg, np.float32)
    repl = {
        "W1": np.ascontiguousarray(W_msg[:H]).astype(BF),
        "W2": np.ascontiguousarray(W_msg[H:2 * H]).astype(BF),
        "wdiag": wdiag.astype(BF),
        "W_res": np.asarray(W_res, dtype=np.float32).astype(BF),
        "Wu1": np.ascontiguousarray(W_upd[:H]).astype(BF),
        "Wu2": np.ascontiguousarray(W_upd[H:]).astype(BF),
        "bupd_col": np.asarray(b_upd, dtype=np.float32).reshape(H, 1),
        "iota_rep": np.tile(iota.reshape(1, 128), (128, 1)).astype(BF),
    }

    emb = np.asarray(node_embed, dtype=np.float32)
    for c in range(n_cores):
        sel, cw = per_core[c]
        # per-tile edge slot assignment (window-major, padded per window)
        rows_pad = np.zeros(T_pad * 128, dtype=np.int64)
        cols_pad = np.zeros(T_pad * 128, dtype=np.int64)
        valid = np.zeros(T_pad * 128, dtype=bool)
        colp = np.full((128, T_pad), -1.0, dtype=np.float32)
        distp = np.zeros((T_pad, 128), dtype=np.float32)
        start = 0
        for w in range(n_win):
            cnt = int(counts[c, w])
            if cnt:
                idx = np.arange(cnt)
                slot = (tile_base[w] + idx // 128) * 128 + idx % 128
                e = sel[start:start + cnt]
                rows_pad[slot] = row[e]
                cols_pad[slot] = col[e]
                valid[slot] = True
                distp.reshape(-1)[slot] = dist[e]
                colp[idx % 128, tile_base[w] + idx // 128] = cw[start:start + cnt]
                start += cnt
        # pre-gathered transposed slabs [ch, T_pad*128]
        srcT = embT[:, rows_pad].copy()
        dstT = embT[:, cols_pad].copy()
        srcT[:, ~valid] = 0
        dstT[:, ~valid] = 0
        # distR: per run [9, 128]: rows 0..7 = tile dists, row 8 = ones
        distR = np.zeros((RMAX + 1, R * 128), dtype=np.float32)
        dr = distp.reshape(R, RMAX, 128)
        for k in range(RMAX):
            distR[k] = dr[:, k, :].reshape(R * 128)
        distR[RMAX] = 1.0
        m = dict(repl)
        shardT = np.zeros((H, S_pad), dtype=BF)
        for j in range(n_win):
            g = blk_at[c, j]
            if g >= 0:
                nb = min(128, N - g * 128)
                shardT[:, j * 128:j * 128 + nb] = \
                    emb[g * 128:g * 128 + nb].T.astype(BF)
        m["emb_shardT"] = shardT
        m["srcT"] = srcT
        m["dstT"] = dstT
        m["distR"] = distR.astype(BF)
        m["colp"] = colp.astype(BF)
        in_maps.append(m)

    cfg = dict(N=N, S=S, S_pad=S_pad, n_win=n_win, R=R, T=T, T_pad=T_pad,
               win_of_tile=win_of_tile, first_wp=first_wp, last_wp=last_wp,
               n_cores=n_cores, blk_at=blk_at)
    return cfg, in_maps


def unshard(cfg, outs):
    """Assemble the full [N, H] output from per-core [H, S_pad] transposed
    slabs laid out in (core, slot) block order."""
    N, n_win, n_cores = cfg["N"], cfg["n_win"], cfg["n_cores"]
    blk_at = cfg["blk_at"]
    out = np.empty((N, H), dtype=np.float32)
    for c in range(n_cores):
        for j in range(n_win):
            g = blk_at[c, j]
            if g >= 0:
                nb = min(128, N - g * 128)
                out[g * 128:g * 128 + nb] = \
                    outs[c][:, j * 128:j * 128 + nb].T
    return out


# --------------------------------------------------------------------------
# device program
# --------------------------------------------------------------------------

def build_program(cfg, debug=False):
    S_pad, n_win, R, T, T_pad = (cfg["S_pad"], cfg["n_win"], cfg["R"],
                                 cfg["T"], cfg["T_pad"])
    win_of_tile = cfg["win_of_tile"]
    first_wp, last_wp = cfg["first_wp"], cfg["last_wp"]

    nc = bacc.Bacc("TRN2", target_bir_lowering=False, debug=debug,
                   num_devices=cfg["n_cores"])

    din = lambda n, s, dt: nc.dram_tensor(n, s, dt, kind="ExternalInput")
    W1 = din("W1", [H, H], BF16)
    W2 = din("W2", [H, H], BF16)
    wdiag = din("wdiag", [RMAX + 1, RMAX * 128], BF16)
    W_res = din("W_res", [H, H], BF16)
    Wu1 = din("Wu1", [H, H], BF16)
    Wu2 = din("Wu2", [H, H], BF16)
    bupd_col = din("bupd_col", [H, 1], F32)
    iota_rep = din("iota_rep", [128, 128], BF16)
    emb_shardT = din("emb_shardT", [H, S_pad], BF16)
    srcT = din("srcT", [H, T_pad * 128], BF16)
    dstT = din("dstT", [H, T_pad * 128], BF16)
    distR = din("distR", [RMAX + 1, R * 128], BF16)
    colp = din("colp", [128, T_pad], BF16)

    out_d = nc.dram_tensor("out", [H, S_pad], F32, kind="ExternalOutput")

    with tile.TileContext(nc) as tc:
        with (
            tc.tile_pool(name="const", bufs=1) as cp,
            tc.tile_pool(name="sb", bufs=2) as sb,
            tc.tile_pool(name="big", bufs=1) as bigp,
            tc.tile_pool(name="ps", bufs=2, space="PSUM") as ps,
            tc.tile_pool(name="aggp", bufs=2, space="PSUM") as aggp,
            tc.tile_pool(name="p3ps", bufs=2, space="PSUM") as p3ps,
        ):
            def cload(t, shape, dt):
                s = cp.tile(shape, dt, tag=t.name)
                nc.sync.dma_start(s[:], t[:])
                return s

            W1s = cload(W1, [H, H], BF16)
            W2s = cload(W2, [H, H], BF16)
            wdiags = cload(wdiag, [RMAX + 1, RMAX * 128], BF16)
            Wress = cload(W_res, [H, H], BF16)
            Wu1s = cload(Wu1, [H, H], BF16)
            Wu2s = cload(Wu2, [H, H], BF16)
            bupds = cload(bupd_col, [H, 1], F32)
            iotars = cload(iota_rep, [128, 128], BF16)
            colps = cload(colp, [128, T_pad], BF16)
            distRs = cload(distR, [RMAX + 1, R * 128], BF16)
            emb_sb = bigp.tile([H, S_pad], BF16, tag="emb_sb")
            nc.sync.dma_start(emb_sb[:], emb_shardT[:])
            aggrT = bigp.tile([128, S_pad], BF16, tag="aggrT")

            # ---------- node update MLP, one 512-col block -----------------
            def emit_p3_block(b0):
                nb = min(512, S_pad - b0)
                ps_u = p3ps.tile([128, 512], F32, tag="p3ps", name="ps_u")
                pu = ps_u[:]
                nc.tensor.matmul(out=pu[:, 0:nb], lhsT=Wu1s[:],
                                 rhs=emb_sb[:, b0:b0 + nb], start=True,
                                 stop=False)
                nc.tensor.matmul(out=pu[:, 0:nb], lhsT=Wu2s[:],
                                 rhs=aggrT[:, b0:b0 + nb], start=False,
                                 stop=True)
                r_sb = sb.tile([128, 512], F32, tag="p3r", name="r_sb")
                nc.scalar.activation(out=r_sb[:, 0:nb], in_=pu[:, 0:nb],
                                     func=mybir.ActivationFunctionType.Relu,
                                     bias=bupds[:])
                ps_r = p3ps.tile([128, 512], F32, tag="p3ps", name="ps_r")
                pr = ps_r[:]
                nc.tensor.matmul(out=pr[:, 0:nb], lhsT=Wress[:],
                                 rhs=emb_sb[:, b0:b0 + nb], start=True,
                                 stop=True)
                o_sb = sb.tile([128, 512], F32, tag="p3o", name="o_sb")
                nc.vector.tensor_tensor(out=o_sb[:, 0:nb], in0=r_sb[:, 0:nb],
                                        in1=pr[:, 0:nb],
                                        op=mybir.AluOpType.add)
                nc.scalar.dma_start(out_d[:, b0:b0 + nb], o_sb[:, 0:nb])

            # window w's aggregate is final after its last tile; map final
            # tiles -> ready P3 blocks
            blocks_after = {}
            for b0 in range(0, S_pad, 512):
                wins = range(b0 // 128, min(b0 + 512, S_pad) // 128)
                fins = [last_wp[w] for w in wins if w in last_wp]
                if fins:
                    blocks_after.setdefault(max(fins), []).append(b0)
            p3_emitted = set()

            # zero windows that never receive edges, before any P3 block runs
            for w in range(n_win):
                if w not in first_wp:
                    nc.vector.memset(aggrT[:, w * 128:(w + 1) * 128], 0.0)

            # ---------- edge loop ------------------------------------------
            # segsum for run ri is emitted after run ri+1's projection
            # matmuls so the PE never stalls on relu (ACT) / o8 (DVE)
            aggr_tiles = {}
            copy_alt = [0]

            def emit_segsum(t0, L, msg_bf, o8):
                for k in range(L):
                    t = t0 + k
                    w = win_of_tile[t]
                    if t == first_wp[w]:
                        aggr_t = aggp.tile([128, H], F32, tag="aggr")
                        aggr_tiles[w] = aggr_t
                    nc.tensor.matmul(out=aggr_tiles[w][:],
                                     lhsT=msg_bf[:, k, :], rhs=o8[:, k, :],
                                     start=(t == first_wp[w]),
                                     stop=(t == last_wp[w]))
                    if t == last_wp[w]:
                        dstw = aggrT[:, w * 128:(w + 1) * 128]
                        if copy_alt[0] % 2:
                            nc.scalar.activation(
                                out=dstw, in_=aggr_tiles[w][:],
                                func=mybir.ActivationFunctionType.Copy)
                        else:
                            nc.vector.tensor_scalar_add(
                                out=dstw, in0=aggr_tiles[w][:], scalar1=0.0)
                        copy_alt[0] += 1
                        del aggr_tiles[w]
                        for b0 in blocks_after.get(t, []):
                            emit_p3_block(b0)
                            p3_emitted.add(b0)

            pending = None
            for ri in range(R):
                t0 = ri * RMAX
                L = min(RMAX, T - t0)

                src_sb = sb.tile([128, RMAX, H], BF16, tag="src", bufs=3)
                nc.sync.dma_start(src_sb[:, 0:L, :],
                                  srcT[:, t0 * 128:(t0 + L) * 128]
                                  .rearrange("p (k e) -> p k e", k=L))
                dst_sb = sb.tile([128, RMAX, H], BF16, tag="dst", bufs=3)
                nc.gpsimd.dma_start(dst_sb[:, 0:L, :],
                                    dstT[:, t0 * 128:(t0 + L) * 128]
                                    .rearrange("p (k e) -> p k e", k=L))

                msg_ps = ps.tile([128, RMAX, H], F32, tag="msgps")
                flat = msg_ps[:].rearrange("p k e -> p (k e)")
                # dist*w_d + b (rank-9, block-diagonal), opens accumulation
                for o in range(0, L * 128, 512):
                    oe = min(o + 512, L * 128)
                    nc.tensor.matmul(out=flat[:, o:oe],
                                     lhsT=distRs[:, ri * 128:(ri + 1) * 128],
                                     rhs=wdiags[:, o:oe], start=True,
                                     stop=False, skip_group_check=True)
                # + src @ W1 + dst @ W2 per tile
                for k in range(L):
                    nc.tensor.matmul(out=msg_ps[:, k, :],
                                     lhsT=src_sb[:, k, :], rhs=W1s[:],
                                     start=False, stop=False,
                                     skip_group_check=True)
                    nc.tensor.matmul(out=msg_ps[:, k, :],
                                     lhsT=dst_sb[:, k, :], rhs=W2s[:],
                                     start=False, stop=True,
                                     skip_group_check=True)
                # previous run's segment-sum (PE waits on its relu no more)
                if pending is not None:
                    emit_segsum(*pending)
                # relu + cast (ACT)
                msg_bf = sb.tile([128, RMAX, H], BF16, tag="msgb", bufs=3)
                nc.scalar.activation(out=msg_bf[:, 0:L, :],
                                     in_=msg_ps[:, 0:L, :],
                                     func=mybir.ActivationFunctionType.Relu)
                # scatter one-hot by local col
                o8 = sb.tile([128, RMAX, H], BF16, tag="o8", bufs=3)
                nc.vector.tensor_tensor(
                    out=o8[:, 0:L, :],
                    in0=colps[:, t0:t0 + L, None].to_broadcast([128, L, 128]),
                    in1=iotars[:, None, :].to_broadcast([128, L, 128]),
                    op=mybir.AluOpType.is_equal)
                pending = (t0, L, msg_bf, o8)

            if pending is not None:
                emit_segsum(*pending)
            for b0 in range(0, S_pad, 512):
                if b0 not in p3_emitted:
                    emit_p3_block(b0)

    nc.compile()
    return nc


# --------------------------------------------------------------------------
# entry point
# --------------------------------------------------------------------------

def kernel(node_embed, node_pos, W_res, W_msg, b_msg, W_upd, b_upd,
           edge_index, n_cores=8, _run=None):
    cfg, in_maps = host_prep(node_embed, node_pos, W_res, W_msg, b_msg,
                             W_upd, b_upd, edge_index, n_cores)
    nc = build_program(cfg)
    if _run is None:
        res = run_bass_kernel_spmd(nc, in_maps, core_ids=list(range(n_cores)))
        outs = [res.results[c]["out"] for c in range(n_cores)]
    else:
        outs = _run(nc, in_maps)
    return unshard(cfg, outs)


# revision 18
# speedup vs baseline: 4.2274x; 1.1368x over previous
"""Trainium2 Bass kernel for an equivariant GNN message-passing layer.

Full inputs in, full output out. 8-way owner-computes sharding by edge target
node (col). The host sorts each core's edges by target window, pads tiles to
128 lanes, and pre-gathers the raw endpoint embedding rows into sequential
slabs (srcT/dstT, transposed [ch, edge]); per-edge squared distances are
precomputed on host. The device computes, per core c (nodes [c*S, (c+1)*S)):

  msg[e]  = relu(src_e @ W1 + dst_e @ W2 + dist_e * w_d + b)   (f32 PSUM)
  aggrT   = one-hot scatter-sum of msg by col                  [128, S_pad]
  outT    = Wres^T emb^T + relu(Wu1^T emb^T + Wu2^T aggrT + b_upd)

with W1 = W_msg[:128], W2 = W_msg[128:256], w_d = W_msg[256]. All matmuls are
bf16 inputs with f32 PSUM accumulation. dist+bias enter via a single rank-9
matmul per 512 edge-columns (8 block-diagonal dist rows + a ones row streaming
[w_d blocks; b tiled]). The output is produced transposed [128, S_pad]; the
host transposes back.
"""

import sys

for _p in ("/opt/trn_rl_repo",):
    if _p not in sys.path:
        sys.path.insert(0, _p)

import numpy as np
import ml_dtypes

import concourse.bacc as bacc
import concourse.bass as bass
import concourse.mybir as mybir
import concourse.tile as tile
from concourse.bass_utils import run_bass_kernel_spmd

F32 = mybir.dt.float32
BF16 = mybir.dt.bfloat16
BF = ml_dtypes.bfloat16

H = 128          # hidden/in channels (hardcoded for this problem)
RMAX = 8         # tiles per run


# --------------------------------------------------------------------------
# host-side prep
# --------------------------------------------------------------------------

def host_prep(node_embed, node_pos, W_res, W_msg, b_msg, W_upd, b_upd,
              edge_index, n_cores):
    N, C_in = node_embed.shape
    assert C_in == H and W_msg.shape == (2 * H + 1, H)
    assert N % n_cores == 0
    S = N // n_cores
    n_win = -(-S // 128)
    S_pad = n_win * 128

    row = np.asarray(edge_index[0], dtype=np.int64)
    col = np.asarray(edge_index[1], dtype=np.int64)
    pos = np.asarray(node_pos, dtype=np.float32)
    diff = pos[row] - pos[col]
    dist = np.sum(diff * diff, axis=1).astype(np.float32)   # [E]

    # global 128-node blocks, assigned to (core, slot) by sorted edge count
    # round-robin so the per-slot max across cores (which sets the padded
    # tile count) tracks the mean instead of the tail
    NB = -(-N // 128)
    assert n_cores * n_win >= NB
    blk_of_edge = col // 128
    cnt_g = np.bincount(blk_of_edge, minlength=NB)
    order = np.argsort(-cnt_g, kind="stable")
    blk_at = np.full((n_cores, n_win), -1, dtype=np.int64)  # (c, slot) -> blk
    core_of_blk = np.zeros(NB, dtype=np.int64)
    slot_of_blk = np.zeros(NB, dtype=np.int64)
    for j in range(n_win):
        for c in range(n_cores):
            i = j * n_cores + c
            if i < NB:
                g = order[i]
                blk_at[c, j] = g
                core_of_blk[g] = c
                slot_of_blk[g] = j

    core_of = core_of_blk[blk_of_edge]

    # per-core edge lists sorted by slot
    per_core = []
    counts = np.zeros((n_cores, n_win), dtype=np.int64)
    for c in range(n_cores):
        sel = np.nonzero(core_of == c)[0]
        w = slot_of_blk[blk_of_edge[sel]]
        order_e = np.argsort(w, kind="stable")
        sel, w = sel[order_e], w[order_e]
        cw = (col[sel] % 128).astype(np.float32)
        np.add.at(counts[c], w, 1)
        per_core.append((sel, cw))

    tiles_w = -(-counts.max(axis=0) // 128)             # [n_win]
    win_of_tile = []
    for w in range(n_win):
        win_of_tile += [w] * int(tiles_w[w])
    T = len(win_of_tile)
    R = -(-T // RMAX)
    T_pad = R * RMAX
    first_wp, last_wp = {}, {}
    for t, w in enumerate(win_of_tile):
        first_wp.setdefault(w, t)
        last_wp[w] = t
    tile_base = {}
    b = 0
    for w in range(n_win):
        tile_base[w] = b
        b += int(tiles_w[w])

    embT = np.ascontiguousarray(np.asarray(node_embed, dtype=np.float32).T
                                ).astype(BF)            # [H, N]

    in_maps = []
    iota = np.arange(128, dtype=np.float32)
    W_msg = np.asarray(W_msg, dtype=np.float32)
    W_upd = np.asarray(W_upd, dtype=np.float32)
    # rank-9 dist+bias rhs: rows 0..7 block-diagonal w_d, row 8 = b tiled
    wdiag = np.zeros((RMAX + 1, RMAX * 128), dtype=np.float32)
    for k in range(RMAX):
        wdiag[k, k * 128:(k + 1) * 128] = W_msg[2 * H]
        wdiag[RMAX, k * 128:(k + 1) * 128] = np.asarray(b_msg, np.float32)
    repl = {
        "W1": np.ascontiguousarray(W_msg[:H]).astype(BF),
        "W2": np.ascontiguousarray(W_msg[H:2 * H]).astype(BF),
        "wdiag": wdiag.astype(BF),
        "W_res": np.asarray(W_res, dtype=np.float32).astype(BF),
        "Wu1": np.ascontiguousarray(W_upd[:H]).astype(BF),
        "Wu2": np.ascontiguousarray(W_upd[H:]).astype(BF),
        "bupd_col": np.asarray(b_upd, dtype=np.float32).reshape(H, 1),
        "iota_rep": np.tile(iota.reshape(1, 128), (128, 1)).astype(BF),
    }

    emb = np.asarray(node_embed, dtype=np.float32)
    for c in range(n_cores):
        sel, cw = per_core[c]
        # per-tile edge slot assignment (window-major, padded per window)
        rows_pad = np.zeros(T_pad * 128, dtype=np.int64)
        cols_pad = np.zeros(T_pad * 128, dtype=np.int64)
        valid = np.zeros(T_pad * 128, dtype=bool)
        colp = np.full((128, T_pad), -1.0, dtype=np.float32)
        distp = np.zeros((T_pad, 128), dtype=np.float32)
        start = 0
        for w in range(n_win):
            cnt = int(counts[c, w])
            if cnt:
                idx = np.arange(cnt)
                slot = (tile_base[w] + idx // 128) * 128 + idx % 128
                e = sel[start:start + cnt]
                rows_pad[slot] = row[e]
                cols_pad[slot] = col[e]
                valid[slot] = True
                distp.reshape(-1)[slot] = dist[e]
                colp[idx % 128, tile_base[w] + idx // 128] = cw[start:start + cnt]
                start += cnt
        # pre-gathered transposed slabs [ch, T_pad*128]
        srcT = embT[:, rows_pad].copy()
        dstT = embT[:, cols_pad].copy()
        srcT[:, ~valid] = 0
        dstT[:, ~valid] = 0
        # distR: per run [9, 128]: rows 0..7 = tile dists, row 8 = ones
        distR = np.zeros((RMAX + 1, R * 128), dtype=np.float32)
        dr = distp.reshape(R, RMAX, 128)
        for k in range(RMAX):
            distR[k] = dr[:, k, :].reshape(R * 128)
        distR[RMAX] = 1.0
        m = dict(repl)
        shardT = np.zeros((H, S_pad), dtype=BF)
        for j in range(n_win):
            g = blk_at[c, j]
            if g >= 0:
                nb = min(128, N - g * 128)
                shardT[:, j * 128:j * 128 + nb] = \
                    emb[g * 128:g * 128 + nb].T.astype(BF)
        m["emb_shardT"] = shardT
        m["srcT"] = srcT
        m["dstT"] = dstT
        m["distR"] = distR.astype(BF)
        m["colp"] = colp.astype(BF)
        in_maps.append(m)

    cfg = dict(N=N, S=S, S_pad=S_pad, n_win=n_win, R=R, T=T, T_pad=T_pad,
               win_of_tile=win_of_tile, first_wp=first_wp, last_wp=last_wp,
               n_cores=n_cores, blk_at=blk_at)
    return cfg, in_maps


def unshard(cfg, outs):
    """Assemble the full [N, H] output from per-core [H, S_pad] transposed
    slabs laid out in (core, slot) block order."""
    N, n_win, n_cores = cfg["N"], cfg["n_win"], cfg["n_cores"]
    blk_at = cfg["blk_at"]
    out = np.empty((N, H), dtype=np.float32)
    for c in range(n_cores):
        for j in range(n_win):
            g = blk_at[c, j]
            if g >= 0:
                nb = min(128, N - g * 128)
                out[g * 128:g * 128 + nb] = \
                    outs[c][:, j * 128:j * 128 + nb].T
    return out


# --------------------------------------------------------------------------
# device program
# --------------------------------------------------------------------------

def build_program(cfg, debug=False):
    S_pad, n_win, R, T, T_pad = (cfg["S_pad"], cfg["n_win"], cfg["R"],
                                 cfg["T"], cfg["T_pad"])
    win_of_tile = cfg["win_of_tile"]
    first_wp, last_wp = cfg["first_wp"], cfg["last_wp"]

    nc = bacc.Bacc("TRN2", target_bir_lowering=False, debug=debug,
                   num_devices=cfg["n_cores"])

    din = lambda n, s, dt: nc.dram_tensor(n, s, dt, kind="ExternalInput")
    W1 = din("W1", [H, H], BF16)
    W2 = din("W2", [H, H], BF16)
    wdiag = din("wdiag", [RMAX + 1, RMAX * 128], BF16)
    W_res = din("W_res", [H, H], BF16)
    Wu1 = din("Wu1", [H, H], BF16)
    Wu2 = din("Wu2", [H, H], BF16)
    bupd_col = din("bupd_col", [H, 1], F32)
    iota_rep = din("iota_rep", [128, 128], BF16)
    emb_shardT = din("emb_shardT", [H, S_pad], BF16)
    srcT = din("srcT", [H, T_pad * 128], BF16)
    dstT = din("dstT", [H, T_pad * 128], BF16)
    distR = din("distR", [RMAX + 1, R * 128], BF16)
    colp = din("colp", [128, T_pad], BF16)

    out_d = nc.dram_tensor("out", [H, S_pad], F32, kind="ExternalOutput")

    with tile.TileContext(nc) as tc:
        with (
            tc.tile_pool(name="const", bufs=1) as cp,
            tc.tile_pool(name="sb", bufs=2) as sb,
            tc.tile_pool(name="big", bufs=1) as bigp,
            tc.tile_pool(name="ps", bufs=2, space="PSUM") as ps,
            tc.tile_pool(name="aggp", bufs=2, space="PSUM") as aggp,
            tc.tile_pool(name="p3ps", bufs=2, space="PSUM") as p3ps,
        ):
            def cload(t, shape, dt, eng=None):
                s = cp.tile(shape, dt, tag=t.name)
                (eng or nc.sync).dma_start(s[:], t[:])
                return s

            # edge-phase-critical consts on the sync queue (ahead of slabs);
            # P3-only consts ride the vector queue so the PE can start early
            W1s = cload(W1, [H, H], BF16)
            W2s = cload(W2, [H, H], BF16)
            wdiags = cload(wdiag, [RMAX + 1, RMAX * 128], BF16)
            iotars = cload(iota_rep, [128, 128], BF16)
            colps = cload(colp, [128, T_pad], BF16)
            distRs = cload(distR, [RMAX + 1, R * 128], BF16)
            Wress = cload(W_res, [H, H], BF16, nc.scalar)
            Wu1s = cload(Wu1, [H, H], BF16, nc.scalar)
            Wu2s = cload(Wu2, [H, H], BF16, nc.scalar)
            bupds = cload(bupd_col, [H, 1], F32, nc.scalar)
            emb_sb = bigp.tile([H, S_pad], BF16, tag="emb_sb")
            nc.scalar.dma_start(emb_sb[:], emb_shardT[:])
            aggrT = bigp.tile([128, S_pad], BF16, tag="aggrT")

            # ---------- node update MLP, one 512-col block -----------------
            def emit_p3_block(b0):
                nb = min(512, S_pad - b0)
                ps_u = p3ps.tile([128, 512], F32, tag="p3ps", name="ps_u")
                pu = ps_u[:]
                nc.tensor.matmul(out=pu[:, 0:nb], lhsT=Wu1s[:],
                                 rhs=emb_sb[:, b0:b0 + nb], start=True,
                                 stop=False)
                nc.tensor.matmul(out=pu[:, 0:nb], lhsT=Wu2s[:],
                                 rhs=aggrT[:, b0:b0 + nb], start=False,
                                 stop=True)
                r_sb = sb.tile([128, 512], F32, tag="p3r", name="r_sb")
                nc.scalar.activation(out=r_sb[:, 0:nb], in_=pu[:, 0:nb],
                                     func=mybir.ActivationFunctionType.Relu,
                                     bias=bupds[:])
                ps_r = p3ps.tile([128, 512], F32, tag="p3ps", name="ps_r")
                pr = ps_r[:]
                nc.tensor.matmul(out=pr[:, 0:nb], lhsT=Wress[:],
                                 rhs=emb_sb[:, b0:b0 + nb], start=True,
                                 stop=True)
                o_sb = sb.tile([128, 512], F32, tag="p3o", name="o_sb")
                nc.vector.tensor_tensor(out=o_sb[:, 0:nb], in0=r_sb[:, 0:nb],
                                        in1=pr[:, 0:nb],
                                        op=mybir.AluOpType.add)
                nc.scalar.dma_start(out_d[:, b0:b0 + nb], o_sb[:, 0:nb])

            # window w's aggregate is final after its last tile; map final
            # tiles -> ready P3 blocks
            blocks_after = {}
            for b0 in range(0, S_pad, 512):
                wins = range(b0 // 128, min(b0 + 512, S_pad) // 128)
                fins = [last_wp[w] for w in wins if w in last_wp]
                if fins:
                    blocks_after.setdefault(max(fins), []).append(b0)
            p3_emitted = set()

            # zero windows that never receive edges, before any P3 block runs
            for w in range(n_win):
                if w not in first_wp:
                    nc.vector.memset(aggrT[:, w * 128:(w + 1) * 128], 0.0)

            # ---------- edge loop ------------------------------------------
            # segsum for run ri is emitted after run ri+1's projection
            # matmuls so the PE never stalls on relu (ACT) / o8 (DVE)
            aggr_tiles = {}
            copy_alt = [0]

            def emit_segsum(t0, L, msg_bf, o8):
                for k in range(L):
                    t = t0 + k
                    w = win_of_tile[t]
                    if t == first_wp[w]:
                        aggr_t = aggp.tile([128, H], F32, tag="aggr")
                        aggr_tiles[w] = aggr_t
                    nc.tensor.matmul(out=aggr_tiles[w][:],
                                     lhsT=msg_bf[:, k, :], rhs=o8[:, k, :],
                                     start=(t == first_wp[w]),
                                     stop=(t == last_wp[w]))
                    if t == last_wp[w]:
                        dstw = aggrT[:, w * 128:(w + 1) * 128]
                        if copy_alt[0] % 2:
                            nc.scalar.activation(
                                out=dstw, in_=aggr_tiles[w][:],
                                func=mybir.ActivationFunctionType.Copy)
                        else:
                            nc.vector.tensor_scalar_add(
                                out=dstw, in0=aggr_tiles[w][:], scalar1=0.0)
                        copy_alt[0] += 1
                        del aggr_tiles[w]
                        for b0 in blocks_after.get(t, []):
                            emit_p3_block(b0)
                            p3_emitted.add(b0)

            pending = None
            for ri in range(R):
                t0 = ri * RMAX
                L = min(RMAX, T - t0)

                src_sb = sb.tile([128, RMAX, H], BF16, tag="src", bufs=4)
                nc.sync.dma_start(src_sb[:, 0:L, :],
                                  srcT[:, t0 * 128:(t0 + L) * 128]
                                  .rearrange("p (k e) -> p k e", k=L))
                dst_sb = sb.tile([128, RMAX, H], BF16, tag="dst", bufs=4)
                nc.gpsimd.dma_start(dst_sb[:, 0:L, :],
                                    dstT[:, t0 * 128:(t0 + L) * 128]
                                    .rearrange("p (k e) -> p k e", k=L))

                msg_ps = ps.tile([128, RMAX, H], F32, tag="msgps")
                flat = msg_ps[:].rearrange("p k e -> p (k e)")
                # dist*w_d + b (rank-9, block-diagonal), opens accumulation
                for o in range(0, L * 128, 512):
                    oe = min(o + 512, L * 128)
                    nc.tensor.matmul(out=flat[:, o:oe],
                                     lhsT=distRs[:, ri * 128:(ri + 1) * 128],
                                     rhs=wdiags[:, o:oe], start=True,
                                     stop=False, skip_group_check=True)
                # + src @ W1 + dst @ W2 per tile
                for k in range(L):
                    nc.tensor.matmul(out=msg_ps[:, k, :],
                                     lhsT=src_sb[:, k, :], rhs=W1s[:],
                                     start=False, stop=False,
                                     skip_group_check=True)
                    nc.tensor.matmul(out=msg_ps[:, k, :],
                                     lhsT=dst_sb[:, k, :], rhs=W2s[:],
                                     start=False, stop=True,
                                     skip_group_check=True)
                # previous run's segment-sum (PE waits on its relu no more)
                if pending is not None:
                    emit_segsum(*pending)
                # relu + cast (ACT)
                msg_bf = sb.tile([128, RMAX, H], BF16, tag="msgb", bufs=3)
                nc.scalar.activation(out=msg_bf[:, 0:L, :],
                                     in_=msg_ps[:, 0:L, :],
                                     func=mybir.ActivationFunctionType.Relu)
                # scatter one-hot by local col
                o8 = sb.tile([128, RMAX, H], BF16, tag="o8", bufs=3)
                nc.vector.tensor_tensor(
                    out=o8[:, 0:L, :],
                    in0=colps[:, t0:t0 + L, None].to_broadcast([128, L, 128]),
                    in1=iotars[:, None, :].to_broadcast([128, L, 128]),
                    op=mybir.AluOpType.is_equal)
                pending = (t0, L, msg_bf, o8)

            if pending is not None:
                emit_segsum(*pending)
            for b0 in range(0, S_pad, 512):
                if b0 not in p3_emitted:
                    emit_p3_block(b0)

    nc.compile()
    return nc


# --------------------------------------------------------------------------
# entry point
# --------------------------------------------------------------------------

def kernel(node_embed, node_pos, W_res, W_msg, b_msg, W_upd, b_upd,
           edge_index, n_cores=8, _run=None):
    cfg, in_maps = host_prep(node_embed, node_pos, W_res, W_msg, b_msg,
                             W_upd, b_upd, edge_index, n_cores)
    nc = build_program(cfg)
    if _run is None:
        res = run_bass_kernel_spmd(nc, in_maps, core_ids=list(range(n_cores)))
        outs = [res.results[c]["out"] for c in range(n_cores)]
    else:
        outs = _run(nc, in_maps)
    return unshard(cfg, outs)
